# revision 80
# baseline (speedup 1.0000x reference)
"""LiteMLA (EfficientViT multi-scale linear attention) Trainium2 Bass kernel.

Sharding: data-parallel over batch B=8 across 8 NeuronCores (1 image/core).
Per-core pipeline (matmul operands bf16, PSUM accumulation fp32):
  1. qkv = Wqkv @ x computed twice with host-permuted weights:
     pass1 (natural channel order) -> zero-padded SBUF image for conv taps,
     pass2 (q|k|v separated order) -> attention Q buffer + id-scale K/V stages.
  2. s3/s5: depthwise 3x3/5x5 + grouped 1x1 FUSED on host into per-tap
     block-diagonal [96,96] weights (4 head-groups per block); PE matmuls
     accumulate taps in PSUM reading shifted slices of the padded image.
  3. relu-linear attention: per spatial tile, relu(k)/v transposed on PE and
     reduced into per-16-head vk outer products (PSUM accumulated over all
     4096 positions); denominator comes from row-sums of relu(k).
  4. vk -> block-diagonal apply weights via host 0/1 masks (no tiny copies);
     out = vk @ relu(q) in dd-major layout so denominators are contiguous;
     normalize with reciprocal + PE broadcast-expand; proj uses a host-padded
     weight with zero rows on denominator positions.

All SBUF operand slices start at partition 0/32/64/96 (HW requirement).
"""

import hashlib
import sys

import numpy as np

sys.path.insert(0, "/opt/trn_rl_repo")

B, CIN, HH, WW = 8, 256, 64, 64
N = HH * WW            # 4096
HEADS = 32             # per scale
C3 = 768
NHEADS = 96
PADW = WW + 4          # 68
NT = 8                 # spatial tiles of 512 positions (8 image rows each)
TN = 512
HALF = 2               # nts processed per conv weight fetch
TAPS3 = [(dy, dx) for dy in (-1, 0, 1) for dx in (-1, 0, 1)]
TAPS5 = [(dy, dx) for dy in (-2, -1, 0, 1, 2) for dx in (-2, -1, 0, 1, 2)]
NBLK = 8               # conv channel blocks of 4 head-groups
BLK = 96
NREG = 6               # vk regions of 16 heads
NAPP = 12              # apply groups of 8 heads

_cache = {}


def _head_of(g12, i):
    return 16 * (g12 // 2) + 8 * (g12 % 2) + i


def _host_weights(inp):
    f32 = np.float32
    W = np.asarray(inp["qkv_w"], f32)[:, :, 0, 0]            # [768, 256]
    qkv_b = np.asarray(inp["qkv_b"], f32)
    pw = {3: np.asarray(inp["pw3_w"], f32)[:, :, 0, 0],
          5: np.asarray(inp["pw5_w"], f32)[:, :, 0, 0]}
    pwb = {3: np.asarray(inp["pw3_b"], f32), 5: np.asarray(inp["pw5_b"], f32)}
    dw = {3: np.asarray(inp["dw3_w"], f32)[:, 0],
          5: np.asarray(inp["dw5_w"], f32)[:, 0]}
    dwb = {3: np.asarray(inp["dw3_b"], f32), 5: np.asarray(inp["dw5_b"], f32)}
    proj_w = np.asarray(inp["proj_w"], f32)[:, :, 0, 0]      # [256, 768]
    proj_b = np.asarray(inp["proj_b"], f32)

    d = {}
    d["w1t"] = np.ascontiguousarray(W.T)                     # [256, 768]
    hh = np.repeat(np.arange(HEADS), 8)
    ee = np.tile(np.arange(8), HEADS)
    perm2 = np.concatenate([hh * 24 + ee, hh * 24 + 8 + ee, hh * 24 + 16 + ee])
    d["w2t"] = np.ascontiguousarray(W[perm2].T)
    d["bi2"] = qkv_b[perm2].reshape(768, 1)

    # fused conv weights: per tap, 8 blocks of 4 groups, [96in, 96out q|k|v]
    oo = np.arange(24)
    for s, taps in ((3, TAPS3), (5, TAPS5)):
        T = len(taps)
        M = pw[s].reshape(32, 24, 24)                        # [g, oo, i]
        dv = dw[s].reshape(32, 24, T)                        # [g, i, t]
        F = np.einsum("goi,git->tgio", M, dv)                # [t, g, i, oo]
        Ft = F.reshape(T, NBLK, 4, 24, 24)
        fw = np.zeros((T, NBLK, 4, 24, BLK), f32)
        bias24 = pwb[s].reshape(32, 24) + np.einsum(
            "goi,gi->go", M, dwb[s].reshape(32, 24))         # [g, oo]
        b24 = bias24.reshape(NBLK, 4, 24)
        fb = np.zeros((NBLK, 4, BLK), f32)
        for gl in range(4):
            m = (oo // 8) * 32 + gl * 8 + (oo % 8)           # [q32|k32|v32]
            fw[:, :, gl, :, m] = np.moveaxis(Ft[:, :, gl], -1, 0)
            fb[:, gl, m] = b24[:, gl]
        # pair-contiguous layout per block: [b, pair, 96, 2*96] so a
        # 2-tap weight fetch is one plain 2D DMA (odd tap counts get a
        # zero-padded, never-read second half in the last pair)
        P2 = (T + 1) // 2
        f4 = fw.reshape(T, NBLK, BLK, BLK)
        fwp = np.zeros((NBLK, P2, BLK, 2 * BLK), f32)
        for t in range(T):
            fwp[:, t // 2, :, (t % 2) * BLK:(t % 2 + 1) * BLK] = f4[t]
        d[f"fw{s}"] = fwp.reshape(NBLK * P2, BLK, 2 * BLK)
        d[f"bc{s}"] = fb.sum(axis=1).reshape(NBLK, BLK, 1)

    # expand lhsT: out row (dd,h) <- recip row h, two half-groups paired in
    # columns 0:64 / 64:128; 2 variants selecting which 16-row group of a
    # 32-row reciprocal block (SBUF partition offsets must be 32-multiples)
    E32 = np.zeros((2, 32, 128), f32)
    for v in range(2):
        for hf in range(2):
            for h in range(8):
                for dd in range(8):
                    E32[v, 16 * v + 8 * hf + h, 64 * hf + 8 * dd + h] = 1.0
    d["exp"] = E32
    d["idt"] = np.eye(128, dtype=f32)

    # masks for vk -> apply-weight assembly (dd-major cols)
    for half in range(2):
        mp = np.zeros((128, 64), f32)   # [(hp,e), (dd,h)]
        md = np.zeros((128, 8), f32)    # [(hp,e), h]
        for p in range(128):
            hp = p // 8
            for h in range(8):
                if hp == h + 8 * half:
                    md[p, h] = 1.0
                    for dd in range(8):
                        mp[p, 8 * dd + h] = 1.0
        d[f"mp{half}"] = mp
        d[f"md{half}"] = md

    # proj lhsT [6, 128, 256]: rows (half, dd, h) for the paired att layout
    g_ = np.arange(NAPP)[:, None]
    i_ = np.arange(8)[None, :]
    Hh = 16 * (g_ // 2) + 8 * (g_ % 2) + i_                  # [12, 8]
    dd_ = np.arange(8)
    cols = 8 * Hh[:, None, :] + dd_[None, :, None]           # [12, dd, i]
    PWm = proj_w.T[cols.reshape(NAPP, 64)]                   # [12, 64, 256]
    d["pw"] = PWm.reshape(6, 128, 256)
    d["pb"] = proj_b.reshape(256, 1)
    return d


def _build():
    import concourse.bass as bass
    import concourse.bacc as bacc_mod
    import concourse.mybir as mybir
    from concourse.tile import TileContext

    dt = mybir.dt
    f32, bf16 = dt.float32, dt.bfloat16
    f32r = dt.float32r
    AF = mybir.ActivationFunctionType
    ALU = mybir.AluOpType
    AX = mybir.AxisListType

    nc = bacc_mod.Bacc()
    # x and the qkv weights need ~fp32 operand precision: bf16 rounding of
    # these two operands alone produces ~0.2 rel err in the final output
    # (heavy cancellation downstream), and fp32r matmuls truncate operands
    # on real HW. So split both into bf16 hi+lo pairs and compute
    # W@x = Wh@xh + Wh@xl + Wl@xh (fp32 PSUM accumulation, wl@xl ~2^-18
    # negligible) at bf16 matmul speed.
    x_h = nc.dram_tensor("xh", [CIN, N], bf16, kind="ExternalInput")
    x_l = nc.dram_tensor("xl", [CIN, N], bf16, kind="ExternalInput")
    dW1h = nc.dram_tensor("w1h", [CIN, C3], bf16, kind="ExternalInput")
    dW2h = nc.dram_tensor("w2h", [CIN, C3], bf16, kind="ExternalInput")
    dW2l = nc.dram_tensor("w2l", [CIN, C3], bf16, kind="ExternalInput")
    dBI2 = nc.dram_tensor("bi2", [C3, 1], f32, kind="ExternalInput")
    dFW3 = nc.dram_tensor("fw3", [5 * NBLK, BLK, 2 * BLK], bf16, kind="ExternalInput")
    dFW5 = nc.dram_tensor("fw5", [13 * NBLK, BLK, 2 * BLK], bf16, kind="ExternalInput")
    dBC3 = nc.dram_tensor("bc3", [NBLK, BLK, 1], f32, kind="ExternalInput")
    dBC5 = nc.dram_tensor("bc5", [NBLK, BLK, 1], f32, kind="ExternalInput")
    dEXP = nc.dram_tensor("exp", [2, 32, 128], bf16, kind="ExternalInput")
    dIDT = nc.dram_tensor("idt", [128, 128], bf16, kind="ExternalInput")
    dMP = [nc.dram_tensor(f"mp{h}", [128, 64], bf16, kind="ExternalInput") for h in range(2)]
    dMD = [nc.dram_tensor(f"md{h}", [128, 8], bf16, kind="ExternalInput") for h in range(2)]
    dPW = nc.dram_tensor("pw", [NREG, 128, 256], bf16, kind="ExternalInput")
    dPB = nc.dram_tensor("pb", [256, 1], f32, kind="ExternalInput")
    d_out = nc.dram_tensor("out", [CIN, N], f32, kind="ExternalOutput")

    with TileContext(nc) as tc:
        with (
            tc.tile_pool(name="consts", bufs=1) as cpool,
            tc.tile_pool(name="persist", bufs=1) as qpool,
            tc.tile_pool(name="wstream", bufs=6) as wpool,
            tc.tile_pool(name="stage", bufs=2) as spool,
            tc.tile_pool(name="psum", bufs=2, space="PSUM") as ppool,
        ):
            # ---- constants ----
            # pass1 (conv-branch qkv) tolerates plain-bf16 operands (the conv
            # branch's contribution to the output error stays ~0.008 total),
            # so only pass2 carries the hi+lo compensated weights
            w1h = [cpool.tile([128, C3], bf16, name=f"w1h_{k}") for k in range(2)]
            for k in range(2):
                nc.sync.dma_start(out=w1h[k][:], in_=dW1h[128 * k:128 * (k + 1), :])
            w2 = {}
            for p, d2 in (("h", dW2h), ("l", dW2l)):
                w2[p] = [cpool.tile([128, C3], bf16, name=f"w2{p}_{k}")
                         for k in range(2)]
                for k in range(2):
                    nc.sync.dma_start(out=w2[p][k][:], in_=d2[128 * k:128 * (k + 1), :])
            bi2 = [cpool.tile([128, 1], f32, name=f"bi2_{j}") for j in range(6)]
            for j in range(6):
                nc.sync.dma_start(out=bi2[j][:], in_=dBI2[128 * j:128 * (j + 1), :])
            bc = {}
            for s, db in ((3, dBC3), (5, dBC5)):
                bc[s] = [cpool.tile([BLK, 1], f32, name=f"bc{s}_{b}") for b in range(NBLK)]
                for b in range(NBLK):
                    nc.sync.dma_start(out=bc[s][b][:], in_=db[b])
            expw = [cpool.tile([32, 128], bf16, name=f"expw_{v}") for v in range(2)]
            for v in range(2):
                nc.sync.dma_start(out=expw[v][:], in_=dEXP[v])
            idt = cpool.tile([128, 128], bf16, name="idt")
            nc.sync.dma_start(out=idt[:], in_=dIDT[:, :])
            mp = [cpool.tile([128, 64], bf16, name=f"mp_{h}") for h in range(2)]
            md = [cpool.tile([128, 8], bf16, name=f"md_{h}") for h in range(2)]
            for h in range(2):
                nc.sync.dma_start(out=mp[h][:], in_=dMP[h][:, :])
                nc.sync.dma_start(out=md[h][:], in_=dMD[h][:, :])
            pwt = [cpool.tile([128, 256], bf16, name=f"pwt_{g}") for g in range(NREG)]
            for g in range(NREG):
                nc.sync.dma_start(out=pwt[g][:], in_=dPW[g])
            pbt = [cpool.tile([128, 1], f32, name=f"pbt_{m}") for m in range(2)]
            for m in range(2):
                nc.sync.dma_start(out=pbt[m][:], in_=dPB[128 * m:128 * (m + 1), :])

            # ---- persistent activations ----
            pad = [qpool.tile([BLK, PADW, PADW], bf16, name=f"pad_{b}") for b in range(NBLK)]
            for b in range(NBLK):
                nc.gpsimd.memset(pad[b][:], 0.0)
            Q = [qpool.tile([128, N], bf16, name=f"Q_{r}") for r in range(NREG)]
            kpart = [qpool.tile([128, NT], f32, name=f"kpart_{r}") for r in range(NREG)]
            vks_sb = [qpool.tile([128, 128], bf16, name=f"vks_{r}") for r in range(NREG)]

            xbt = {p: [qpool.tile([128, N], bf16, name=f"x{p}_{k}")
                       for k in range(2)] for p in ("h", "l")}
            # nt-major order so pass1's first matmuls aren't waiting on the
            # tail of a p/k-major DMA stream
            for nt in range(NT):
                for p, dx in (("h", x_h), ("l", x_l)):
                    for k in range(2):
                        nc.sync.dma_start(
                            out=xbt[p][k][:, TN * nt:TN * (nt + 1)],
                            in_=dx[128 * k:128 * (k + 1), TN * nt:TN * (nt + 1)])

            def xb(p, k, nt):
                return xbt[p][k][:, TN * nt:TN * (nt + 1)]

            # (w, x) pairs for the compensated qkv product
            QKV_TERMS = (("h", "h"), ("h", "l"), ("l", "h"))

            # ================ pass 1: natural order -> padded image =========
            for b in range(NBLK):
                for nt in range(NT):
                    ps = ppool.tile([BLK, 8, WW], f32, name="ps1", tag="mm")
                    for k in range(2):
                        nc.tensor.matmul(
                            ps[:], w1h[k][:, BLK * b:BLK * (b + 1)],
                            xb("h", k, nt).rearrange("p (a c) -> p a c", c=WW),
                            start=(k == 0), stop=(k == 1))
                    nc.scalar.copy(out=pad[b][:, 2 + 8 * nt:10 + 8 * nt, 2:2 + WW], in_=ps[:])

            # ============ shared per-tile attention stage ====================
            def new_vkp(s_idx):
                return [ppool.tile([128, 128], f32, name=f"vkp_{s_idx}_{t}",
                                   tag="vk", bufs=2) for t in range(2)]

            def process_stage(s_idx, nt, ks, vs, vkp):
                """ks/vs: 2 bf16 [128,512] stage tiles (relu'd k / raw v)."""
                for t in range(2):
                    r = 2 * s_idx + t
                    nc.vector.reduce_sum(out=kpart[r][:, nt:nt + 1], in_=ks[t][:], axis=AX.X)
                for jj in range(4):
                    kT = spool.tile([128, 256], bf16, name="kT", tag="kT", bufs=2)
                    vT = spool.tile([128, 256], bf16, name="vT", tag="vT", bufs=2)
                    for t in range(2):
                        tp = ppool.tile([128, 128], bf16, name="tp", tag="mm")
                        nc.tensor.transpose(tp[:], ks[t][:, 128 * jj:128 * (jj + 1)], idt[:])
                        nc.scalar.copy(out=kT[:, 128 * t:128 * (t + 1)], in_=tp[:])
                        tp2 = ppool.tile([128, 128], bf16, name="tp2", tag="mm")
                        nc.tensor.transpose(tp2[:], vs[t][:, 128 * jj:128 * (jj + 1)], idt[:])
                        nc.vector.tensor_copy(out=vT[:, 128 * t:128 * (t + 1)], in_=tp2[:])
                    first = (nt == 0 and jj == 0)
                    last = (nt == NT - 1 and jj == 3)
                    for t in range(2):
                        nc.tensor.matmul(
                            vkp[t][:],
                            kT[:, 128 * t:128 * (t + 1)], vT[:, 128 * t:128 * (t + 1)],
                            start=first, stop=last)
                if nt == NT - 1:
                    for t in range(2):
                        nc.scalar.copy(out=vks_sb[2 * s_idx + t][:], in_=vkp[t][:])

            # ========== pass 2: separated order -> Q + id-scale k/v ==========
            vkp_id = new_vkp(0)
            for nt in range(NT):
                ks, vs = [None, None], [None, None]
                for j in range(6):
                    ps = ppool.tile([128, TN], f32, name="ps2", tag="mm")
                    for i, (pw_, px_) in enumerate(QKV_TERMS):
                        for k in range(2):
                            nc.tensor.matmul(ps[:], w2[pw_][k][:, 128 * j:128 * (j + 1)],
                                             xb(px_, k, nt),
                                             start=(i == 0 and k == 0),
                                             stop=(i == 2 and k == 1))
                    if j < 2:
                        nc.scalar.activation(out=Q[j][:, TN * nt:TN * (nt + 1)], in_=ps[:],
                                             func=AF.Relu, bias=bi2[j][:], scale=1.0)
                    elif j < 4:
                        t = j - 2
                        kst = spool.tile([128, TN], bf16, name="ks", tag=f"ks{t}", bufs=2)
                        nc.scalar.activation(out=kst[:], in_=ps[:], func=AF.Relu,
                                             bias=bi2[j][:], scale=1.0)
                        ks[t] = kst
                    else:
                        t = j - 4
                        vst = spool.tile([128, TN], bf16, name="vs", tag=f"vs{t}", bufs=2)
                        nc.vector.tensor_scalar(out=vst[:], in0=ps[:], scalar1=bi2[j][:],
                                                scalar2=None, op0=ALU.add)
                        vs[t] = vst
                process_stage(0, nt, ks, vs, vkp_id)

            # ================= fused conv scales =============================
            for s, taps, dfw, s_idx in ((3, TAPS3, dFW3, 1), (5, TAPS5, dFW5, 2)):
                vkp_s = new_vkp(s_idx)
                for h0 in range(0, NT, HALF):
                    stg = {}
                    for nth in range(HALF):
                        for t in range(2):
                            stg[("k", nth, t)] = spool.tile(
                                [128, TN], bf16, name="ks", tag=f"ks{t}", bufs=2)
                            stg[("v", nth, t)] = spool.tile(
                                [128, TN], bf16, name="vs", tag=f"vs{t}", bufs=2)
                    for b in range(NBLK):
                        # bufs=3: block b+1's accumulators must not wait on
                        # block b's PSUM eviction (was a 2.7us PE stall/block);
                        # 3 (not 4) frees one 2KB PSUM bank for the dps tag
                        cps = [ppool.tile([BLK, 8, WW], f32, name="cp",
                                          tag="conv", bufs=3)
                               for _ in range(HALF)]
                        # fetch 2 taps per DMA trigger on the (otherwise idle)
                        # sync engine: per-(tap,block) gpsimd triggers used to
                        # occupy GpSimd ~660us, pacing the whole conv phase
                        npair = (len(taps) + 1) // 2
                        fwt2 = None
                        for ti, (dy, dx) in enumerate(taps):
                            if ti % 2 == 0:
                                fwt2 = wpool.tile([BLK, 2 * BLK], bf16,
                                                  name="fwt", tag="fw")
                                nc.sync.dma_start(
                                    out=fwt2[:], in_=dfw[b * npair + ti // 2])
                            fwt = fwt2[:, BLK * (ti % 2):BLK * (ti % 2 + 1)]
                            for nth in range(HALF):
                                nt = h0 + nth
                                nc.tensor.matmul(
                                    cps[nth][:], fwt,
                                    pad[b][:, 2 + 8 * nt + dy:10 + 8 * nt + dy,
                                           2 + dx:2 + dx + WW],
                                    start=(ti == 0), stop=(ti == len(taps) - 1))
                        qt, qr = (256 * s_idx + 32 * b) // 128, (32 * b) % 128
                        t2, r2 = b // 4, (32 * b) % 128
                        for nth in range(HALF):
                            nt = h0 + nth
                            cp = cps[nth]
                            nc.scalar.activation(
                                out=Q[qt][qr:qr + 32, TN * nt:TN * (nt + 1)],
                                in_=cp[0:32].rearrange("p a c -> p (a c)"),
                                func=AF.Relu, bias=bc[s][b][0:32, :], scale=1.0)
                            nc.scalar.activation(
                                out=stg[("k", nth, t2)][r2:r2 + 32, :],
                                in_=cp[32:64].rearrange("p a c -> p (a c)"),
                                func=AF.Relu, bias=bc[s][b][32:64, :], scale=1.0)
                            nc.vector.tensor_scalar(
                                out=stg[("v", nth, t2)][r2:r2 + 32, :],
                                in0=cp[64:96].rearrange("p a c -> p (a c)"),
                                scalar1=bc[s][b][64:96, :], scalar2=None, op0=ALU.add)
                    for nth in range(HALF):
                        process_stage(s_idx, h0 + nth,
                                      [stg[("k", nth, t)] for t in range(2)],
                                      [stg[("v", nth, t)] for t in range(2)], vkp_s)

            # ============== assemble apply weights from vk ===================
            apw2 = []
            denw = []
            for r in range(NREG):
                kf = qpool.tile([128, 1], f32, name=f"kfin_{r}")
                nc.vector.reduce_sum(out=kf[:], in_=kpart[r][:], axis=AX.X)
                vks = vks_sb[r]
                # den weights for quad-packed den matmuls: [128, 32] with this
                # region's two halves in columns 16*(r%2)..+16, zeros elsewhere
                dnw = qpool.tile([128, 32], bf16, name=f"denw_{r}")
                nc.gpsimd.memset(dnw[:], 0.0)
                # paired apply weights: both halves of the region in one
                # [128, 128] lhsT (one apply matmul per region per tile)
                aw2 = qpool.tile([128, 128], bf16, name=f"apw2_{r}")
                for half in range(2):
                    nc.vector.tensor_tensor(
                        out=aw2[:, 64 * half:64 * (half + 1)].rearrange(
                            "p (d h) -> p d h", h=8),
                        in0=vks[:, 64 * half:64 * (half + 1)].rearrange(
                            "p (h d) -> p d h", d=8),
                        in1=mp[half][:].rearrange("p (d h) -> p d h", h=8),
                        op=ALU.mult)
                    nc.vector.tensor_scalar(
                        out=dnw[:, 16 * (r % 2) + 8 * half:16 * (r % 2) + 8 * half + 8],
                        in0=md[half][:], scalar1=kf[:], scalar2=None, op0=ALU.mult)
                apw2.append(aw2)
                denw.append(dnw)

            # ================= apply + normalize + proj ======================
            for nt in range(NT):
                pjs = [ppool.tile([128, TN], f32, name=f"pj{m}", tag="conv", bufs=3)
                       for m in range(2)]
                # pass A: all 12 denominators -> one batched reciprocal
                # (per-group [8,512] reciprocal chains cost ~190us of vector
                # time and serialized against PE)
                den12 = spool.tile([96, TN], f32, name="den12", tag="den", bufs=2)
                for G in range(3):
                    # own PSUM tag: with tag "mm" the first den matmul of tile
                    # nt waits ~3us for tile nt-1's aps/eps buffers to drain
                    dps = ppool.tile([32, TN], f32, name="dps", tag="dps", bufs=1)
                    for rr in range(2):
                        r = 2 * G + rr
                        nc.tensor.matmul(dps[:], denw[r][:],
                                         Q[r][:, TN * nt:TN * (nt + 1)],
                                         start=(rr == 0), stop=(rr == 1))
                    nc.scalar.copy(out=den12[32 * G:32 * (G + 1), :], in_=dps[:])
                nc.vector.tensor_scalar(out=den12[:], in0=den12[:], scalar1=1e-15,
                                        scalar2=None, op0=ALU.add)
                rc12 = spool.tile([96, TN], f32, name="rc12", tag="rc", bufs=1)
                scr12 = spool.tile([96, TN], f32, name="scr12", tag="scr", bufs=1)
                nc.vector.reciprocal_approx_accurate(out=rc12[:], in_=den12[:],
                                                     scratch=scr12[:])
                # three base-0 tiles: matmul rhs must share base partition
                # with its lhsT (expw variants live at base 0)
                rcb32 = []
                for G in range(3):
                    rt = spool.tile([32, TN], bf16, name=f"rcb{G}", tag="rcb", bufs=3)
                    nc.scalar.copy(out=rt[:], in_=rc12[32 * G:32 * (G + 1), :])
                    rcb32.append(rt)
                # pass B: apply -> normalize -> proj, both halves of a region
                # paired into single [128,128]-lhsT matmuls
                for j in range(NREG):
                    aps = ppool.tile([128, TN], f32, name="aps", tag="mm")
                    nc.tensor.matmul(aps[:], apw2[j][:], Q[j][:, TN * nt:TN * (nt + 1)],
                                     start=True, stop=True)
                    eps = ppool.tile([128, TN], f32, name="eps", tag="mm")
                    nc.tensor.matmul(eps[:], expw[j % 2][:], rcb32[j // 2][:],
                                     start=True, stop=True)
                    exb = spool.tile([128, TN], f32, name="exb", tag="exb", bufs=2)
                    nc.scalar.copy(out=exb[:], in_=eps[:])
                    at = spool.tile([128, TN], bf16, name="at", tag="at", bufs=2)
                    nc.vector.tensor_tensor(out=at[:], in0=aps[:], in1=exb[:], op=ALU.mult)
                    for m in range(2):
                        nc.tensor.matmul(pjs[m][:], pwt[j][:, 128 * m:128 * (m + 1)],
                                         at[:], start=(j == 0), stop=(j == NREG - 1))
                for m in range(2):
                    ob = spool.tile([128, TN], f32, name="ob", tag="ob", bufs=2)
                    nc.vector.tensor_scalar(out=ob[:], in0=pjs[m][:], scalar1=pbt[m][:],
                                            scalar2=None, op0=ALU.add)
                    nc.sync.dma_start(
                        out=d_out[128 * m:128 * (m + 1), TN * nt:TN * (nt + 1)], in_=ob[:])
    return nc


def _get_nc():
    if "nc" not in _cache:
        nc = _build()
        nc.compile()
        _cache["nc"] = nc
    return _cache["nc"]


def _whash(inputs):
    h = hashlib.blake2b(digest_size=16)
    for name in ("qkv_w", "qkv_b", "dw3_w", "dw3_b", "pw3_w", "pw3_b",
                 "dw5_w", "dw5_b", "pw5_w", "pw5_b", "proj_w", "proj_b"):
        h.update(np.ascontiguousarray(np.asarray(inputs[name], np.float32)))
    return h.hexdigest()


def _feeds(inputs):
    import ml_dtypes

    def bf(a):
        return np.asarray(a, ml_dtypes.bfloat16)

    def split(a):
        hi = bf(a)
        lo = bf(np.asarray(a, np.float32) - np.asarray(hi, np.float32))
        return hi, lo

    key = _whash(inputs)
    if _cache.get("feeds_key") != key:
        d = _host_weights(inputs)
        w2h, w2l = split(d["w2t"])
        base = {
            "w1h": bf(d["w1t"]), "w2h": w2h, "w2l": w2l,
            "bi2": d["bi2"].astype(np.float32),
            "fw3": bf(d["fw3"]), "fw5": bf(d["fw5"]),
            "bc3": d["bc3"].astype(np.float32), "bc5": d["bc5"].astype(np.float32),
            "exp": bf(d["exp"]), "idt": bf(d["idt"]),
            "mp0": bf(d["mp0"]), "mp1": bf(d["mp1"]),
            "md0": bf(d["md0"]), "md1": bf(d["md1"]),
            "pw": bf(d["pw"]), "pb": d["pb"].astype(np.float32),
        }
        _cache["feeds_key"] = key
        _cache["feeds_base"] = base
    x = np.asarray(inputs["x"], np.float32).reshape(B, CIN, N)
    xh, xl = split(x)
    return _cache["feeds_base"], (xh, xl)


def _get_runner():
    """Build the jitted shard_map callable once; reuse across kernel() calls.

    Mirrors concourse.bass2jax.run_bass_via_pjrt but caches the jitted
    function (avoids re-lowering/re-compiling the XLA wrapper per call) and
    keeps the replicated weight operands device-resident.
    """
    if "runner" in _cache:
        return _cache["runner"]
    import jax
    import concourse.mybir as mybir
    from concourse import bass2jax
    from jax.experimental.shard_map import shard_map
    from jax.sharding import Mesh, PartitionSpec

    bass2jax.install_neuronx_cc_hook()
    nc = _get_nc()
    assert nc.dbg_addr is None or not nc.dbg_callbacks

    partition_name = (nc.partition_id_tensor.name
                      if nc.partition_id_tensor else None)
    in_names, out_names, out_avals = [], [], []
    for alloc in nc.m.functions[0].allocations:
        if not isinstance(alloc, mybir.MemoryLocationSet):
            continue
        name = alloc.memorylocations[0].name
        if alloc.kind == "ExternalInput":
            if name != partition_name:
                in_names.append(name)
        elif alloc.kind == "ExternalOutput":
            out_names.append(name)
            out_avals.append(jax.core.ShapedArray(
                tuple(alloc.tensor_shape), mybir.dt.np(alloc.dtype)))
    n_params = len(in_names)
    all_in = in_names + out_names + ([partition_name] if partition_name else [])

    def _body(*args):
        operands = list(args)
        if partition_name is not None:
            operands.append(bass2jax.partition_id_tensor())
        return tuple(bass2jax._bass_exec_p.bind(
            *operands,
            out_avals=tuple(out_avals),
            in_names=tuple(all_in),
            out_names=tuple(out_names),
            lowering_input_output_aliases=(),
            sim_require_finite=True,
            sim_require_nnan=True,
            nc=nc,
        ))

    devices = jax.devices()[:B]
    mesh = Mesh(np.asarray(devices), ("core",))
    sharded = jax.jit(
        shard_map(_body, mesh=mesh,
                  in_specs=(PartitionSpec("core"),) * (n_params + len(out_names)),
                  out_specs=(PartitionSpec("core"),) * len(out_names),
                  check_rep=False),
        keep_unused=True)
    _cache["runner"] = (sharded, in_names, out_names, out_avals, mesh)
    return _cache["runner"]


def kernel(**inputs):
    import jax
    from jax.sharding import NamedSharding, PartitionSpec

    base, (xh, xl) = _feeds(inputs)
    sharded, in_names, out_names, out_avals, mesh = _get_runner()

    sh = NamedSharding(mesh, PartitionSpec("core"))
    key = _cache["feeds_key"]
    if _cache.get("dev_key") != key:
        dev = {}
        for name in in_names:
            if name in ("xh", "xl"):
                continue
            a = np.asarray(base[name])
            rep = np.concatenate([a] * B, axis=0)
            dev[name] = jax.device_put(rep, sh)
        _cache["dev_key"] = key
        _cache["dev_weights"] = dev
    dev = _cache["dev_weights"]

    if "dev_zeros" not in _cache:
        _cache["dev_zeros"] = [
            jax.device_put(
                np.zeros((B * av.shape[0],) + tuple(av.shape[1:]), av.dtype), sh)
            for av in out_avals]

    xg = {"xh": xh, "xl": xl}
    args = []
    for name in in_names:
        if name in ("xh", "xl"):
            args.append(np.ascontiguousarray(xg[name].reshape(B * CIN, N)))
        else:
            args.append(dev[name])
    args.extend(_cache["dev_zeros"])

    out_arrs = sharded(*args)
    idx = out_names.index("out")
    out = np.asarray(out_arrs[idx]).reshape(B, CIN, HH, WW)
    return out.astype(np.float32)



# revision 83
# speedup vs baseline: 1.0820x; 1.0820x over previous
"""LiteMLA (EfficientViT multi-scale linear attention) Trainium2 Bass kernel.

Sharding: data-parallel over batch B=8 across 8 NeuronCores (1 image/core).
Per-core pipeline (matmul operands bf16, PSUM accumulation fp32):
  1. qkv = Wqkv @ x computed twice with host-permuted weights:
     pass1 (natural channel order) -> zero-padded SBUF image for conv taps,
     pass2 (q|k|v separated order) -> attention Q buffer + id-scale K/V stages.
  2. s3/s5: depthwise 3x3/5x5 + grouped 1x1 FUSED on host into per-tap
     block-diagonal [96,96] weights (4 head-groups per block); PE matmuls
     accumulate taps in PSUM reading shifted slices of the padded image.
  3. relu-linear attention: per spatial tile, relu(k)/v transposed on PE and
     reduced into per-16-head vk outer products (PSUM accumulated over all
     4096 positions); denominator comes from row-sums of relu(k).
  4. vk -> block-diagonal apply weights via host 0/1 masks (no tiny copies);
     out = vk @ relu(q) in dd-major layout so denominators are contiguous;
     normalize with reciprocal + PE broadcast-expand; proj uses a host-padded
     weight with zero rows on denominator positions.

All SBUF operand slices start at partition 0/32/64/96 (HW requirement).
"""

import hashlib
import sys

import numpy as np

sys.path.insert(0, "/opt/trn_rl_repo")

B, CIN, HH, WW = 8, 256, 64, 64
N = HH * WW            # 4096
HEADS = 32             # per scale
C3 = 768
NHEADS = 96
PADW = WW + 4          # 68
NT = 8                 # spatial tiles of 512 positions (8 image rows each)
TN = 512
HALF = 2               # nts processed per conv weight fetch
TAPS3 = [(dy, dx) for dy in (-1, 0, 1) for dx in (-1, 0, 1)]
TAPS5 = [(dy, dx) for dy in (-2, -1, 0, 1, 2) for dx in (-2, -1, 0, 1, 2)]
NBLK = 8               # conv channel blocks of 4 head-groups
BLK = 96
NREG = 6               # vk regions of 16 heads
NAPP = 12              # apply groups of 8 heads

_cache = {}


def _head_of(g12, i):
    return 16 * (g12 // 2) + 8 * (g12 % 2) + i


def _host_weights(inp):
    f32 = np.float32
    W = np.asarray(inp["qkv_w"], f32)[:, :, 0, 0]            # [768, 256]
    qkv_b = np.asarray(inp["qkv_b"], f32)
    pw = {3: np.asarray(inp["pw3_w"], f32)[:, :, 0, 0],
          5: np.asarray(inp["pw5_w"], f32)[:, :, 0, 0]}
    pwb = {3: np.asarray(inp["pw3_b"], f32), 5: np.asarray(inp["pw5_b"], f32)}
    dw = {3: np.asarray(inp["dw3_w"], f32)[:, 0],
          5: np.asarray(inp["dw5_w"], f32)[:, 0]}
    dwb = {3: np.asarray(inp["dw3_b"], f32), 5: np.asarray(inp["dw5_b"], f32)}
    proj_w = np.asarray(inp["proj_w"], f32)[:, :, 0, 0]      # [256, 768]
    proj_b = np.asarray(inp["proj_b"], f32)

    d = {}
    d["w1t"] = np.ascontiguousarray(W.T)                     # [256, 768]
    hh = np.repeat(np.arange(HEADS), 8)
    ee = np.tile(np.arange(8), HEADS)
    perm2 = np.concatenate([hh * 24 + ee, hh * 24 + 8 + ee, hh * 24 + 16 + ee])
    d["w2t"] = np.ascontiguousarray(W[perm2].T)
    d["bi2"] = qkv_b[perm2].reshape(768, 1)

    # fused conv weights: per tap, 8 blocks of 4 groups, [96in, 96out q|k|v]
    oo = np.arange(24)
    for s, taps in ((3, TAPS3), (5, TAPS5)):
        T = len(taps)
        M = pw[s].reshape(32, 24, 24)                        # [g, oo, i]
        dv = dw[s].reshape(32, 24, T)                        # [g, i, t]
        F = np.einsum("goi,git->tgio", M, dv)                # [t, g, i, oo]
        Ft = F.reshape(T, NBLK, 4, 24, 24)
        fw = np.zeros((T, NBLK, 4, 24, BLK), f32)
        bias24 = pwb[s].reshape(32, 24) + np.einsum(
            "goi,gi->go", M, dwb[s].reshape(32, 24))         # [g, oo]
        b24 = bias24.reshape(NBLK, 4, 24)
        fb = np.zeros((NBLK, 4, BLK), f32)
        for gl in range(4):
            m = (oo // 8) * 32 + gl * 8 + (oo % 8)           # [q32|k32|v32]
            fw[:, :, gl, :, m] = np.moveaxis(Ft[:, :, gl], -1, 0)
            fb[:, gl, m] = b24[:, gl]
        # pair-contiguous layout per block: [b, pair, 96, 2*96] so a
        # 2-tap weight fetch is one plain 2D DMA (odd tap counts get a
        # zero-padded, never-read second half in the last pair)
        P2 = (T + 1) // 2
        f4 = fw.reshape(T, NBLK, BLK, BLK)
        fwp = np.zeros((NBLK, P2, BLK, 2 * BLK), f32)
        for t in range(T):
            fwp[:, t // 2, :, (t % 2) * BLK:(t % 2 + 1) * BLK] = f4[t]
        d[f"fw{s}"] = fwp.reshape(NBLK * P2, BLK, 2 * BLK)
        d[f"bc{s}"] = fb.sum(axis=1).reshape(NBLK, BLK, 1)

    # expand lhsT: out row (dd,h) <- recip row h, two half-groups paired in
    # columns 0:64 / 64:128; 2 variants selecting which 16-row group of a
    # 32-row reciprocal block (SBUF partition offsets must be 32-multiples)
    E32 = np.zeros((2, 32, 128), f32)
    for v in range(2):
        for hf in range(2):
            for h in range(8):
                for dd in range(8):
                    E32[v, 16 * v + 8 * hf + h, 64 * hf + 8 * dd + h] = 1.0
    d["exp"] = E32
    d["idt"] = np.eye(128, dtype=f32)

    # masks for vk -> apply-weight assembly (dd-major cols)
    for half in range(2):
        mp = np.zeros((128, 64), f32)   # [(hp,e), (dd,h)]
        md = np.zeros((128, 8), f32)    # [(hp,e), h]
        for p in range(128):
            hp = p // 8
            for h in range(8):
                if hp == h + 8 * half:
                    md[p, h] = 1.0
                    for dd in range(8):
                        mp[p, 8 * dd + h] = 1.0
        d[f"mp{half}"] = mp
        d[f"md{half}"] = md

    # proj lhsT [6, 128, 256]: rows (half, dd, h) for the paired att layout
    g_ = np.arange(NAPP)[:, None]
    i_ = np.arange(8)[None, :]
    Hh = 16 * (g_ // 2) + 8 * (g_ % 2) + i_                  # [12, 8]
    dd_ = np.arange(8)
    cols = 8 * Hh[:, None, :] + dd_[None, :, None]           # [12, dd, i]
    PWm = proj_w.T[cols.reshape(NAPP, 64)]                   # [12, 64, 256]
    d["pw"] = PWm.reshape(6, 128, 256)
    d["pb"] = proj_b.reshape(256, 1)
    return d


def _build():
    import concourse.bass as bass
    import concourse.bacc as bacc_mod
    import concourse.mybir as mybir
    from concourse.tile import TileContext

    dt = mybir.dt
    f32, bf16 = dt.float32, dt.bfloat16
    f32r = dt.float32r
    AF = mybir.ActivationFunctionType
    ALU = mybir.AluOpType
    AX = mybir.AxisListType

    nc = bacc_mod.Bacc()
    # x and the qkv weights need ~fp32 operand precision: bf16 rounding of
    # these two operands alone produces ~0.2 rel err in the final output
    # (heavy cancellation downstream), and fp32r matmuls truncate operands
    # on real HW. So split both into bf16 hi+lo pairs and compute
    # W@x = Wh@xh + Wh@xl + Wl@xh (fp32 PSUM accumulation, wl@xl ~2^-18
    # negligible) at bf16 matmul speed.
    x_h = nc.dram_tensor("xh", [CIN, N], bf16, kind="ExternalInput")
    x_l = nc.dram_tensor("xl", [CIN, N], bf16, kind="ExternalInput")
    dW1h = nc.dram_tensor("w1h", [CIN, C3], bf16, kind="ExternalInput")
    dW2h = nc.dram_tensor("w2h", [CIN, C3], bf16, kind="ExternalInput")
    dW2l = nc.dram_tensor("w2l", [CIN, C3], bf16, kind="ExternalInput")
    dBI2 = nc.dram_tensor("bi2", [C3, 1], f32, kind="ExternalInput")
    dFW3 = nc.dram_tensor("fw3", [5 * NBLK, BLK, 2 * BLK], bf16, kind="ExternalInput")
    dFW5 = nc.dram_tensor("fw5", [13 * NBLK, BLK, 2 * BLK], bf16, kind="ExternalInput")
    dBC3 = nc.dram_tensor("bc3", [NBLK, BLK, 1], f32, kind="ExternalInput")
    dBC5 = nc.dram_tensor("bc5", [NBLK, BLK, 1], f32, kind="ExternalInput")
    dEXP = nc.dram_tensor("exp", [2, 32, 128], bf16, kind="ExternalInput")
    dIDT = nc.dram_tensor("idt", [128, 128], bf16, kind="ExternalInput")
    dMP = [nc.dram_tensor(f"mp{h}", [128, 64], bf16, kind="ExternalInput") for h in range(2)]
    dMD = [nc.dram_tensor(f"md{h}", [128, 8], bf16, kind="ExternalInput") for h in range(2)]
    dPW = nc.dram_tensor("pw", [NREG, 128, 256], bf16, kind="ExternalInput")
    dPB = nc.dram_tensor("pb", [256, 1], f32, kind="ExternalInput")
    d_out = nc.dram_tensor("out", [CIN, N], f32, kind="ExternalOutput")

    with TileContext(nc) as tc:
        with (
            tc.tile_pool(name="consts", bufs=1) as cpool,
            tc.tile_pool(name="persist", bufs=1) as qpool,
            tc.tile_pool(name="wstream", bufs=6) as wpool,
            tc.tile_pool(name="stage", bufs=2) as spool,
            tc.tile_pool(name="psum", bufs=2, space="PSUM") as ppool,
        ):
            # ---- constants ----
            # pass1 (conv-branch qkv) tolerates plain-bf16 operands (the conv
            # branch's contribution to the output error stays ~0.008 total),
            # so only pass2 carries the hi+lo compensated weights
            w1h = [cpool.tile([128, C3], bf16, name=f"w1h_{k}") for k in range(2)]
            for k in range(2):
                nc.sync.dma_start(out=w1h[k][:], in_=dW1h[128 * k:128 * (k + 1), :])
            w2 = {}
            for p, d2 in (("h", dW2h), ("l", dW2l)):
                w2[p] = [cpool.tile([128, C3], bf16, name=f"w2{p}_{k}")
                         for k in range(2)]
                for k in range(2):
                    nc.sync.dma_start(out=w2[p][k][:], in_=d2[128 * k:128 * (k + 1), :])
            bi2 = [cpool.tile([128, 1], f32, name=f"bi2_{j}") for j in range(6)]
            for j in range(6):
                nc.sync.dma_start(out=bi2[j][:], in_=dBI2[128 * j:128 * (j + 1), :])
            bc = {}
            for s, db in ((3, dBC3), (5, dBC5)):
                bc[s] = [cpool.tile([BLK, 1], f32, name=f"bc{s}_{b}") for b in range(NBLK)]
                for b in range(NBLK):
                    nc.sync.dma_start(out=bc[s][b][:], in_=db[b])
            expw = [cpool.tile([32, 128], bf16, name=f"expw_{v}") for v in range(2)]
            for v in range(2):
                nc.sync.dma_start(out=expw[v][:], in_=dEXP[v])
            idt = cpool.tile([128, 128], bf16, name="idt")
            nc.sync.dma_start(out=idt[:], in_=dIDT[:, :])
            mp = [cpool.tile([128, 64], bf16, name=f"mp_{h}") for h in range(2)]
            md = [cpool.tile([128, 8], bf16, name=f"md_{h}") for h in range(2)]
            for h in range(2):
                nc.sync.dma_start(out=mp[h][:], in_=dMP[h][:, :])
                nc.sync.dma_start(out=md[h][:], in_=dMD[h][:, :])
            pwt = [cpool.tile([128, 256], bf16, name=f"pwt_{g}") for g in range(NREG)]
            for g in range(NREG):
                nc.sync.dma_start(out=pwt[g][:], in_=dPW[g])
            pbt = [cpool.tile([128, 1], f32, name=f"pbt_{m}") for m in range(2)]
            for m in range(2):
                nc.sync.dma_start(out=pbt[m][:], in_=dPB[128 * m:128 * (m + 1), :])

            # ---- persistent activations ----
            pad = [qpool.tile([BLK, PADW, PADW], bf16, name=f"pad_{b}") for b in range(NBLK)]
            for b in range(NBLK):
                nc.gpsimd.memset(pad[b][:], 0.0)
            Q = [qpool.tile([128, N], bf16, name=f"Q_{r}") for r in range(NREG)]
            kpart = [qpool.tile([128, NT], f32, name=f"kpart_{r}") for r in range(NREG)]
            vks_sb = [qpool.tile([128, 128], bf16, name=f"vks_{r}") for r in range(NREG)]

            xbt = {p: [qpool.tile([128, N], bf16, name=f"x{p}_{k}")
                       for k in range(2)] for p in ("h", "l")}
            # nt-major order so pass1's first matmuls aren't waiting on the
            # tail of a p/k-major DMA stream
            for nt in range(NT):
                for p, dx in (("h", x_h), ("l", x_l)):
                    for k in range(2):
                        nc.sync.dma_start(
                            out=xbt[p][k][:, TN * nt:TN * (nt + 1)],
                            in_=dx[128 * k:128 * (k + 1), TN * nt:TN * (nt + 1)])

            def xb(p, k, nt):
                return xbt[p][k][:, TN * nt:TN * (nt + 1)]

            # (w, x) pairs for the compensated qkv product
            QKV_TERMS = (("h", "h"), ("h", "l"), ("l", "h"))

            # ================ pass 1: natural order -> padded image =========
            for b in range(NBLK):
                for nt in range(NT):
                    ps = ppool.tile([BLK, 8, WW], f32, name="ps1", tag="mm")
                    for k in range(2):
                        nc.tensor.matmul(
                            ps[:], w1h[k][:, BLK * b:BLK * (b + 1)],
                            xb("h", k, nt).rearrange("p (a c) -> p a c", c=WW),
                            start=(k == 0), stop=(k == 1))
                    nc.scalar.copy(out=pad[b][:, 2 + 8 * nt:10 + 8 * nt, 2:2 + WW], in_=ps[:])

            # ============ shared per-tile attention stage ====================
            def new_vkp(s_idx):
                return [ppool.tile([128, 128], f32, name=f"vkp_{s_idx}_{t}",
                                   tag="vk", bufs=2) for t in range(2)]

            def process_stage(s_idx, nt, ks, vs, vkp):
                """ks/vs: 2 bf16 [128,512] stage tiles (relu'd k / raw v)."""
                for t in range(2):
                    r = 2 * s_idx + t
                    nc.vector.reduce_sum(out=kpart[r][:, nt:nt + 1], in_=ks[t][:], axis=AX.X)
                for jj in range(4):
                    kT = spool.tile([128, 256], bf16, name="kT", tag="kT", bufs=2)
                    vT = spool.tile([128, 256], bf16, name="vT", tag="vT", bufs=2)
                    for t in range(2):
                        tp = ppool.tile([128, 128], bf16, name="tp", tag="mm")
                        nc.tensor.transpose(tp[:], ks[t][:, 128 * jj:128 * (jj + 1)], idt[:])
                        nc.scalar.copy(out=kT[:, 128 * t:128 * (t + 1)], in_=tp[:])
                        tp2 = ppool.tile([128, 128], bf16, name="tp2", tag="mm")
                        nc.tensor.transpose(tp2[:], vs[t][:, 128 * jj:128 * (jj + 1)], idt[:])
                        nc.vector.tensor_copy(out=vT[:, 128 * t:128 * (t + 1)], in_=tp2[:])
                    first = (nt == 0 and jj == 0)
                    last = (nt == NT - 1 and jj == 3)
                    for t in range(2):
                        nc.tensor.matmul(
                            vkp[t][:],
                            kT[:, 128 * t:128 * (t + 1)], vT[:, 128 * t:128 * (t + 1)],
                            start=first, stop=last)
                if nt == NT - 1:
                    for t in range(2):
                        nc.scalar.copy(out=vks_sb[2 * s_idx + t][:], in_=vkp[t][:])

            # ========== pass 2: separated order -> Q + id-scale k/v ==========
            vkp_id = new_vkp(0)
            for nt in range(NT):
                ks, vs = [None, None], [None, None]
                for j in range(6):
                    ps = ppool.tile([128, TN], f32, name="ps2", tag="mm")
                    for i, (pw_, px_) in enumerate(QKV_TERMS):
                        for k in range(2):
                            nc.tensor.matmul(ps[:], w2[pw_][k][:, 128 * j:128 * (j + 1)],
                                             xb(px_, k, nt),
                                             start=(i == 0 and k == 0),
                                             stop=(i == 2 and k == 1))
                    if j < 2:
                        nc.scalar.activation(out=Q[j][:, TN * nt:TN * (nt + 1)], in_=ps[:],
                                             func=AF.Relu, bias=bi2[j][:], scale=1.0)
                    elif j < 4:
                        t = j - 2
                        kst = spool.tile([128, TN], bf16, name="ks", tag=f"ks{t}", bufs=2)
                        nc.scalar.activation(out=kst[:], in_=ps[:], func=AF.Relu,
                                             bias=bi2[j][:], scale=1.0)
                        ks[t] = kst
                    else:
                        t = j - 4
                        vst = spool.tile([128, TN], bf16, name="vs", tag=f"vs{t}", bufs=2)
                        nc.vector.tensor_scalar(out=vst[:], in0=ps[:], scalar1=bi2[j][:],
                                                scalar2=None, op0=ALU.add)
                        vs[t] = vst
                process_stage(0, nt, ks, vs, vkp_id)

            # ================= fused conv scales =============================
            for s, taps, dfw, s_idx in ((3, TAPS3, dFW3, 1), (5, TAPS5, dFW5, 2)):
                vkp_s = new_vkp(s_idx)
                for h0 in range(0, NT, HALF):
                    stg = {}
                    for nth in range(HALF):
                        for t in range(2):
                            stg[("k", nth, t)] = spool.tile(
                                [128, TN], bf16, name="ks", tag=f"ks{t}", bufs=2)
                            stg[("v", nth, t)] = spool.tile(
                                [128, TN], bf16, name="vs", tag=f"vs{t}", bufs=2)
                    for b in range(NBLK):
                        # bufs=4: block b+1's accumulators must not wait on
                        # block b's PSUM eviction (was a 2.7us PE stall/block)
                        cps = [ppool.tile([BLK, 8, WW], f32, name="cp",
                                          tag="conv", bufs=4)
                               for _ in range(HALF)]
                        # fetch 2 taps per DMA trigger on the (otherwise idle)
                        # sync engine: per-(tap,block) gpsimd triggers used to
                        # occupy GpSimd ~660us, pacing the whole conv phase
                        npair = (len(taps) + 1) // 2
                        fwt2 = None
                        for ti, (dy, dx) in enumerate(taps):
                            if ti % 2 == 0:
                                fwt2 = wpool.tile([BLK, 2 * BLK], bf16,
                                                  name="fwt", tag="fw")
                                nc.sync.dma_start(
                                    out=fwt2[:], in_=dfw[b * npair + ti // 2])
                            fwt = fwt2[:, BLK * (ti % 2):BLK * (ti % 2 + 1)]
                            for nth in range(HALF):
                                nt = h0 + nth
                                nc.tensor.matmul(
                                    cps[nth][:], fwt,
                                    pad[b][:, 2 + 8 * nt + dy:10 + 8 * nt + dy,
                                           2 + dx:2 + dx + WW],
                                    start=(ti == 0), stop=(ti == len(taps) - 1))
                        qt, qr = (256 * s_idx + 32 * b) // 128, (32 * b) % 128
                        t2, r2 = b // 4, (32 * b) % 128
                        for nth in range(HALF):
                            nt = h0 + nth
                            cp = cps[nth]
                            nc.scalar.activation(
                                out=Q[qt][qr:qr + 32, TN * nt:TN * (nt + 1)],
                                in_=cp[0:32].rearrange("p a c -> p (a c)"),
                                func=AF.Relu, bias=bc[s][b][0:32, :], scale=1.0)
                            nc.scalar.activation(
                                out=stg[("k", nth, t2)][r2:r2 + 32, :],
                                in_=cp[32:64].rearrange("p a c -> p (a c)"),
                                func=AF.Relu, bias=bc[s][b][32:64, :], scale=1.0)
                            nc.vector.tensor_scalar(
                                out=stg[("v", nth, t2)][r2:r2 + 32, :],
                                in0=cp[64:96].rearrange("p a c -> p (a c)"),
                                scalar1=bc[s][b][64:96, :], scalar2=None, op0=ALU.add)
                    for nth in range(HALF):
                        process_stage(s_idx, h0 + nth,
                                      [stg[("k", nth, t)] for t in range(2)],
                                      [stg[("v", nth, t)] for t in range(2)], vkp_s)

            # ============== assemble apply weights from vk ===================
            apw2 = []
            denw = []
            for r in range(NREG):
                kf = qpool.tile([128, 1], f32, name=f"kfin_{r}")
                nc.vector.reduce_sum(out=kf[:], in_=kpart[r][:], axis=AX.X)
                vks = vks_sb[r]
                # den weights for quad-packed den matmuls: [128, 32] with this
                # region's two halves in columns 16*(r%2)..+16, zeros elsewhere
                dnw = qpool.tile([128, 32], bf16, name=f"denw_{r}")
                nc.gpsimd.memset(dnw[:], 0.0)
                # paired apply weights: both halves of the region in one
                # [128, 128] lhsT (one apply matmul per region per tile)
                aw2 = qpool.tile([128, 128], bf16, name=f"apw2_{r}")
                for half in range(2):
                    nc.vector.tensor_tensor(
                        out=aw2[:, 64 * half:64 * (half + 1)].rearrange(
                            "p (d h) -> p d h", h=8),
                        in0=vks[:, 64 * half:64 * (half + 1)].rearrange(
                            "p (h d) -> p d h", d=8),
                        in1=mp[half][:].rearrange("p (d h) -> p d h", h=8),
                        op=ALU.mult)
                    nc.vector.tensor_scalar(
                        out=dnw[:, 16 * (r % 2) + 8 * half:16 * (r % 2) + 8 * half + 8],
                        in0=md[half][:], scalar1=kf[:], scalar2=None, op0=ALU.mult)
                apw2.append(aw2)
                denw.append(dnw)

            # ================= apply + normalize + proj ======================
            for nt in range(NT):
                pjs = [ppool.tile([128, TN], f32, name=f"pj{m}", tag="conv", bufs=4)
                       for m in range(2)]
                # pass A: all 12 denominators -> one batched reciprocal
                # (per-group [8,512] reciprocal chains cost ~190us of vector
                # time and serialized against PE)
                den12 = spool.tile([96, TN], f32, name="den12", tag="den", bufs=2)
                for G in range(3):
                    dps = ppool.tile([32, TN], f32, name="dps", tag="mm")
                    for rr in range(2):
                        r = 2 * G + rr
                        nc.tensor.matmul(dps[:], denw[r][:],
                                         Q[r][:, TN * nt:TN * (nt + 1)],
                                         start=(rr == 0), stop=(rr == 1))
                    nc.scalar.copy(out=den12[32 * G:32 * (G + 1), :], in_=dps[:])
                nc.vector.tensor_scalar(out=den12[:], in0=den12[:], scalar1=1e-15,
                                        scalar2=None, op0=ALU.add)
                rc12 = spool.tile([96, TN], f32, name="rc12", tag="rc", bufs=1)
                scr12 = spool.tile([96, TN], f32, name="scr12", tag="scr", bufs=1)
                nc.vector.reciprocal_approx_accurate(out=rc12[:], in_=den12[:],
                                                     scratch=scr12[:])
                # three base-0 tiles: matmul rhs must share base partition
                # with its lhsT (expw variants live at base 0)
                rcb32 = []
                for G in range(3):
                    rt = spool.tile([32, TN], bf16, name=f"rcb{G}", tag="rcb", bufs=3)
                    nc.scalar.copy(out=rt[:], in_=rc12[32 * G:32 * (G + 1), :])
                    rcb32.append(rt)
                # pass B: apply -> normalize -> proj, both halves of a region
                # paired into single [128,128]-lhsT matmuls
                for j in range(NREG):
                    aps = ppool.tile([128, TN], f32, name="aps", tag="mm")
                    nc.tensor.matmul(aps[:], apw2[j][:], Q[j][:, TN * nt:TN * (nt + 1)],
                                     start=True, stop=True)
                    eps = ppool.tile([128, TN], f32, name="eps", tag="mm")
                    nc.tensor.matmul(eps[:], expw[j % 2][:], rcb32[j // 2][:],
                                     start=True, stop=True)
                    exb = spool.tile([128, TN], f32, name="exb", tag="exb", bufs=2)
                    nc.scalar.copy(out=exb[:], in_=eps[:])
                    at = spool.tile([128, TN], bf16, name="at", tag="at", bufs=2)
                    nc.vector.tensor_tensor(out=at[:], in0=aps[:], in1=exb[:], op=ALU.mult)
                    for m in range(2):
                        nc.tensor.matmul(pjs[m][:], pwt[j][:, 128 * m:128 * (m + 1)],
                                         at[:], start=(j == 0), stop=(j == NREG - 1))
                for m in range(2):
                    ob = spool.tile([128, TN], f32, name="ob", tag="ob", bufs=2)
                    nc.vector.tensor_scalar(out=ob[:], in0=pjs[m][:], scalar1=pbt[m][:],
                                            scalar2=None, op0=ALU.add)
                    nc.sync.dma_start(
                        out=d_out[128 * m:128 * (m + 1), TN * nt:TN * (nt + 1)], in_=ob[:])
    return nc


def _get_nc():
    if "nc" not in _cache:
        nc = _build()
        nc.compile()
        _cache["nc"] = nc
    return _cache["nc"]


def _whash(inputs):
    h = hashlib.blake2b(digest_size=16)
    for name in ("qkv_w", "qkv_b", "dw3_w", "dw3_b", "pw3_w", "pw3_b",
                 "dw5_w", "dw5_b", "pw5_w", "pw5_b", "proj_w", "proj_b"):
        h.update(np.ascontiguousarray(np.asarray(inputs[name], np.float32)))
    return h.hexdigest()


def _feeds(inputs):
    import ml_dtypes

    def bf(a):
        return np.asarray(a, ml_dtypes.bfloat16)

    def split(a):
        hi = bf(a)
        lo = bf(np.asarray(a, np.float32) - np.asarray(hi, np.float32))
        return hi, lo

    key = _whash(inputs)
    if _cache.get("feeds_key") != key:
        d = _host_weights(inputs)
        w2h, w2l = split(d["w2t"])
        base = {
            "w1h": bf(d["w1t"]), "w2h": w2h, "w2l": w2l,
            "bi2": d["bi2"].astype(np.float32),
            "fw3": bf(d["fw3"]), "fw5": bf(d["fw5"]),
            "bc3": d["bc3"].astype(np.float32), "bc5": d["bc5"].astype(np.float32),
            "exp": bf(d["exp"]), "idt": bf(d["idt"]),
            "mp0": bf(d["mp0"]), "mp1": bf(d["mp1"]),
            "md0": bf(d["md0"]), "md1": bf(d["md1"]),
            "pw": bf(d["pw"]), "pb": d["pb"].astype(np.float32),
        }
        _cache["feeds_key"] = key
        _cache["feeds_base"] = base
    x = np.asarray(inputs["x"], np.float32).reshape(B, CIN, N)
    xh, xl = split(x)
    return _cache["feeds_base"], (xh, xl)


def _get_runner():
    """Build the jitted shard_map callable once; reuse across kernel() calls.

    Mirrors concourse.bass2jax.run_bass_via_pjrt but caches the jitted
    function (avoids re-lowering/re-compiling the XLA wrapper per call) and
    keeps the replicated weight operands device-resident.
    """
    if "runner" in _cache:
        return _cache["runner"]
    import jax
    import concourse.mybir as mybir
    from concourse import bass2jax
    from jax.experimental.shard_map import shard_map
    from jax.sharding import Mesh, PartitionSpec

    bass2jax.install_neuronx_cc_hook()
    nc = _get_nc()
    assert nc.dbg_addr is None or not nc.dbg_callbacks

    partition_name = (nc.partition_id_tensor.name
                      if nc.partition_id_tensor else None)
    in_names, out_names, out_avals = [], [], []
    for alloc in nc.m.functions[0].allocations:
        if not isinstance(alloc, mybir.MemoryLocationSet):
            continue
        name = alloc.memorylocations[0].name
        if alloc.kind == "ExternalInput":
            if name != partition_name:
                in_names.append(name)
        elif alloc.kind == "ExternalOutput":
            out_names.append(name)
            out_avals.append(jax.core.ShapedArray(
                tuple(alloc.tensor_shape), mybir.dt.np(alloc.dtype)))
    n_params = len(in_names)
    all_in = in_names + out_names + ([partition_name] if partition_name else [])

    def _body(*args):
        operands = list(args)
        if partition_name is not None:
            operands.append(bass2jax.partition_id_tensor())
        return tuple(bass2jax._bass_exec_p.bind(
            *operands,
            out_avals=tuple(out_avals),
            in_names=tuple(all_in),
            out_names=tuple(out_names),
            lowering_input_output_aliases=(),
            sim_require_finite=True,
            sim_require_nnan=True,
            nc=nc,
        ))

    devices = jax.devices()[:B]
    mesh = Mesh(np.asarray(devices), ("core",))
    sharded = jax.jit(
        shard_map(_body, mesh=mesh,
                  in_specs=(PartitionSpec("core"),) * (n_params + len(out_names)),
                  out_specs=(PartitionSpec("core"),) * len(out_names),
                  check_rep=False),
        keep_unused=True)
    _cache["runner"] = (sharded, in_names, out_names, out_avals, mesh)
    return _cache["runner"]


def kernel(**inputs):
    import jax
    from jax.sharding import NamedSharding, PartitionSpec

    base, (xh, xl) = _feeds(inputs)
    sharded, in_names, out_names, out_avals, mesh = _get_runner()

    sh = NamedSharding(mesh, PartitionSpec("core"))
    key = _cache["feeds_key"]
    if _cache.get("dev_key") != key:
        dev = {}
        for name in in_names:
            if name in ("xh", "xl"):
                continue
            a = np.asarray(base[name])
            rep = np.concatenate([a] * B, axis=0)
            dev[name] = jax.device_put(rep, sh)
        _cache["dev_key"] = key
        _cache["dev_weights"] = dev
    dev = _cache["dev_weights"]

    if "dev_zeros" not in _cache:
        _cache["dev_zeros"] = [
            jax.device_put(
                np.zeros((B * av.shape[0],) + tuple(av.shape[1:]), av.dtype), sh)
            for av in out_avals]

    xg = {"xh": xh, "xl": xl}
    args = []
    for name in in_names:
        if name in ("xh", "xl"):
            args.append(np.ascontiguousarray(xg[name].reshape(B * CIN, N)))
        else:
            args.append(dev[name])
    args.extend(_cache["dev_zeros"])

    out_arrs = sharded(*args)
    idx = out_names.index("out")
    out = np.asarray(out_arrs[idx]).reshape(B, CIN, HH, WW)
    return out.astype(np.float32)



# revision 84
# speedup vs baseline: 1.1399x; 1.0535x over previous
"""LiteMLA (EfficientViT multi-scale linear attention) Trainium2 Bass kernel.

Sharding: data-parallel over batch B=8 across 8 NeuronCores (1 image/core).
Per-core pipeline (matmul operands bf16, PSUM accumulation fp32):
  1. qkv = Wqkv @ x computed twice with host-permuted weights:
     pass1 (natural channel order) -> zero-padded SBUF image for conv taps,
     pass2 (q|k|v separated order) -> attention Q buffer + id-scale K/V stages.
  2. s3/s5: depthwise 3x3/5x5 + grouped 1x1 FUSED on host into per-tap
     block-diagonal [96,96] weights (4 head-groups per block); PE matmuls
     accumulate taps in PSUM reading shifted slices of the padded image.
  3. relu-linear attention: per spatial tile, relu(k)/v transposed on PE and
     reduced into per-16-head vk outer products (PSUM accumulated over all
     4096 positions); denominator comes from row-sums of relu(k).
  4. vk -> block-diagonal apply weights via host 0/1 masks (no tiny copies);
     out = vk @ relu(q) in dd-major layout so denominators are contiguous;
     normalize with reciprocal + PE broadcast-expand; proj uses a host-padded
     weight with zero rows on denominator positions.

All SBUF operand slices start at partition 0/32/64/96 (HW requirement).
"""

import hashlib
import sys

import numpy as np

sys.path.insert(0, "/opt/trn_rl_repo")

B, CIN, HH, WW = 8, 256, 64, 64
N = HH * WW            # 4096
HEADS = 32             # per scale
C3 = 768
NHEADS = 96
PADW = WW + 4          # 68
NT = 8                 # spatial tiles of 512 positions (8 image rows each)
TN = 512
HALF = 2               # nts processed per conv weight fetch
TAPS3 = [(dy, dx) for dy in (-1, 0, 1) for dx in (-1, 0, 1)]
TAPS5 = [(dy, dx) for dy in (-2, -1, 0, 1, 2) for dx in (-2, -1, 0, 1, 2)]
NBLK = 8               # conv channel blocks of 4 head-groups
BLK = 96
NREG = 6               # vk regions of 16 heads
NAPP = 12              # apply groups of 8 heads

_cache = {}


def _head_of(g12, i):
    return 16 * (g12 // 2) + 8 * (g12 % 2) + i


def _host_weights(inp):
    f32 = np.float32
    W = np.asarray(inp["qkv_w"], f32)[:, :, 0, 0]            # [768, 256]
    qkv_b = np.asarray(inp["qkv_b"], f32)
    pw = {3: np.asarray(inp["pw3_w"], f32)[:, :, 0, 0],
          5: np.asarray(inp["pw5_w"], f32)[:, :, 0, 0]}
    pwb = {3: np.asarray(inp["pw3_b"], f32), 5: np.asarray(inp["pw5_b"], f32)}
    dw = {3: np.asarray(inp["dw3_w"], f32)[:, 0],
          5: np.asarray(inp["dw5_w"], f32)[:, 0]}
    dwb = {3: np.asarray(inp["dw3_b"], f32), 5: np.asarray(inp["dw5_b"], f32)}
    proj_w = np.asarray(inp["proj_w"], f32)[:, :, 0, 0]      # [256, 768]
    proj_b = np.asarray(inp["proj_b"], f32)

    d = {}
    d["w1t"] = np.ascontiguousarray(W.T)                     # [256, 768]
    hh = np.repeat(np.arange(HEADS), 8)
    ee = np.tile(np.arange(8), HEADS)
    perm2 = np.concatenate([hh * 24 + ee, hh * 24 + 8 + ee, hh * 24 + 16 + ee])
    d["w2t"] = np.ascontiguousarray(W[perm2].T)
    d["bi2"] = qkv_b[perm2].reshape(768, 1)

    # fused conv weights: per tap, 8 blocks of 4 groups, [96in, 96out q|k|v]
    oo = np.arange(24)
    for s, taps in ((3, TAPS3), (5, TAPS5)):
        T = len(taps)
        M = pw[s].reshape(32, 24, 24)                        # [g, oo, i]
        dv = dw[s].reshape(32, 24, T)                        # [g, i, t]
        F = np.einsum("goi,git->tgio", M, dv)                # [t, g, i, oo]
        Ft = F.reshape(T, NBLK, 4, 24, 24)
        fw = np.zeros((T, NBLK, 4, 24, BLK), f32)
        bias24 = pwb[s].reshape(32, 24) + np.einsum(
            "goi,gi->go", M, dwb[s].reshape(32, 24))         # [g, oo]
        b24 = bias24.reshape(NBLK, 4, 24)
        fb = np.zeros((NBLK, 4, BLK), f32)
        for gl in range(4):
            m = (oo // 8) * 32 + gl * 8 + (oo % 8)           # [q32|k32|v32]
            fw[:, :, gl, :, m] = np.moveaxis(Ft[:, :, gl], -1, 0)
            fb[:, gl, m] = b24[:, gl]
        # pair-contiguous layout per block: [b, pair, 96, 2*96] so a
        # 2-tap weight fetch is one plain 2D DMA (odd tap counts get a
        # zero-padded, never-read second half in the last pair)
        P2 = (T + 1) // 2
        f4 = fw.reshape(T, NBLK, BLK, BLK)
        fwp = np.zeros((NBLK, P2, BLK, 2 * BLK), f32)
        for t in range(T):
            fwp[:, t // 2, :, (t % 2) * BLK:(t % 2 + 1) * BLK] = f4[t]
        d[f"fw{s}"] = fwp.reshape(NBLK * P2, BLK, 2 * BLK)
        d[f"bc{s}"] = fb.sum(axis=1).reshape(NBLK, BLK, 1)

    # expand lhsT: out row (dd,h) <- recip row h, two half-groups paired in
    # columns 0:64 / 64:128; 2 variants selecting which 16-row group of a
    # 32-row reciprocal block (SBUF partition offsets must be 32-multiples)
    E32 = np.zeros((2, 32, 128), f32)
    for v in range(2):
        for hf in range(2):
            for h in range(8):
                for dd in range(8):
                    E32[v, 16 * v + 8 * hf + h, 64 * hf + 8 * dd + h] = 1.0
    d["exp"] = E32
    d["idt"] = np.eye(128, dtype=f32)

    # masks for vk -> apply-weight assembly (dd-major cols)
    for half in range(2):
        mp = np.zeros((128, 64), f32)   # [(hp,e), (dd,h)]
        md = np.zeros((128, 8), f32)    # [(hp,e), h]
        for p in range(128):
            hp = p // 8
            for h in range(8):
                if hp == h + 8 * half:
                    md[p, h] = 1.0
                    for dd in range(8):
                        mp[p, 8 * dd + h] = 1.0
        d[f"mp{half}"] = mp
        d[f"md{half}"] = md

    # proj lhsT [6, 128, 256]: rows (half, dd, h) for the paired att layout
    g_ = np.arange(NAPP)[:, None]
    i_ = np.arange(8)[None, :]
    Hh = 16 * (g_ // 2) + 8 * (g_ % 2) + i_                  # [12, 8]
    dd_ = np.arange(8)
    cols = 8 * Hh[:, None, :] + dd_[None, :, None]           # [12, dd, i]
    PWm = proj_w.T[cols.reshape(NAPP, 64)]                   # [12, 64, 256]
    d["pw"] = PWm.reshape(6, 128, 256)
    d["pb"] = proj_b.reshape(256, 1)
    return d


def _build():
    import concourse.bass as bass
    import concourse.bacc as bacc_mod
    import concourse.mybir as mybir
    from concourse.tile import TileContext

    dt = mybir.dt
    f32, bf16 = dt.float32, dt.bfloat16
    f32r = dt.float32r
    AF = mybir.ActivationFunctionType
    ALU = mybir.AluOpType
    AX = mybir.AxisListType

    nc = bacc_mod.Bacc()
    # x and the qkv weights need ~fp32 operand precision: bf16 rounding of
    # these two operands alone produces ~0.2 rel err in the final output
    # (heavy cancellation downstream), and fp32r matmuls truncate operands
    # on real HW. So split both into bf16 hi+lo pairs and compute
    # W@x = Wh@xh + Wh@xl + Wl@xh (fp32 PSUM accumulation, wl@xl ~2^-18
    # negligible) at bf16 matmul speed.
    x_h = nc.dram_tensor("xh", [CIN, N], bf16, kind="ExternalInput")
    x_l = nc.dram_tensor("xl", [CIN, N], bf16, kind="ExternalInput")
    dW1h = nc.dram_tensor("w1h", [CIN, C3], bf16, kind="ExternalInput")
    dW2h = nc.dram_tensor("w2h", [CIN, C3], bf16, kind="ExternalInput")
    dW2l = nc.dram_tensor("w2l", [CIN, C3], bf16, kind="ExternalInput")
    dBI2 = nc.dram_tensor("bi2", [C3, 1], f32, kind="ExternalInput")
    dFW3 = nc.dram_tensor("fw3", [5 * NBLK, BLK, 2 * BLK], bf16, kind="ExternalInput")
    dFW5 = nc.dram_tensor("fw5", [13 * NBLK, BLK, 2 * BLK], bf16, kind="ExternalInput")
    dBC3 = nc.dram_tensor("bc3", [NBLK, BLK, 1], f32, kind="ExternalInput")
    dBC5 = nc.dram_tensor("bc5", [NBLK, BLK, 1], f32, kind="ExternalInput")
    dEXP = nc.dram_tensor("exp", [2, 32, 128], bf16, kind="ExternalInput")
    dIDT = nc.dram_tensor("idt", [128, 128], bf16, kind="ExternalInput")
    dMP = [nc.dram_tensor(f"mp{h}", [128, 64], bf16, kind="ExternalInput") for h in range(2)]
    dMD = [nc.dram_tensor(f"md{h}", [128, 8], bf16, kind="ExternalInput") for h in range(2)]
    dPW = nc.dram_tensor("pw", [NREG, 128, 256], bf16, kind="ExternalInput")
    dPB = nc.dram_tensor("pb", [256, 1], f32, kind="ExternalInput")
    d_out = nc.dram_tensor("out", [CIN, N], f32, kind="ExternalOutput")

    with TileContext(nc) as tc:
        with (
            tc.tile_pool(name="consts", bufs=1) as cpool,
            tc.tile_pool(name="persist", bufs=1) as qpool,
            tc.tile_pool(name="wstream", bufs=6) as wpool,
            tc.tile_pool(name="stage", bufs=2) as spool,
            tc.tile_pool(name="psum", bufs=2, space="PSUM") as ppool,
        ):
            # ---- constants ----
            # pass1 (conv-branch qkv) tolerates plain-bf16 operands (the conv
            # branch's contribution to the output error stays ~0.008 total),
            # so only pass2 carries the hi+lo compensated weights
            w1h = [cpool.tile([128, C3], bf16, name=f"w1h_{k}") for k in range(2)]
            for k in range(2):
                nc.sync.dma_start(out=w1h[k][:], in_=dW1h[128 * k:128 * (k + 1), :])
            w2 = {}
            for p, d2 in (("h", dW2h), ("l", dW2l)):
                w2[p] = [cpool.tile([128, C3], bf16, name=f"w2{p}_{k}")
                         for k in range(2)]
                for k in range(2):
                    nc.sync.dma_start(out=w2[p][k][:], in_=d2[128 * k:128 * (k + 1), :])
            bi2 = [cpool.tile([128, 1], f32, name=f"bi2_{j}") for j in range(6)]
            for j in range(6):
                nc.sync.dma_start(out=bi2[j][:], in_=dBI2[128 * j:128 * (j + 1), :])
            bc = {}
            for s, db in ((3, dBC3), (5, dBC5)):
                bc[s] = [cpool.tile([BLK, 1], f32, name=f"bc{s}_{b}") for b in range(NBLK)]
                for b in range(NBLK):
                    nc.sync.dma_start(out=bc[s][b][:], in_=db[b])
            expw = [cpool.tile([32, 128], bf16, name=f"expw_{v}") for v in range(2)]
            for v in range(2):
                nc.sync.dma_start(out=expw[v][:], in_=dEXP[v])
            idt = cpool.tile([128, 128], bf16, name="idt")
            nc.sync.dma_start(out=idt[:], in_=dIDT[:, :])
            mp = [cpool.tile([128, 64], bf16, name=f"mp_{h}") for h in range(2)]
            md = [cpool.tile([128, 8], bf16, name=f"md_{h}") for h in range(2)]
            for h in range(2):
                nc.sync.dma_start(out=mp[h][:], in_=dMP[h][:, :])
                nc.sync.dma_start(out=md[h][:], in_=dMD[h][:, :])
            pwt = [cpool.tile([128, 256], bf16, name=f"pwt_{g}") for g in range(NREG)]
            for g in range(NREG):
                nc.sync.dma_start(out=pwt[g][:], in_=dPW[g])
            pbt = [cpool.tile([128, 1], f32, name=f"pbt_{m}") for m in range(2)]
            for m in range(2):
                nc.sync.dma_start(out=pbt[m][:], in_=dPB[128 * m:128 * (m + 1), :])

            # ---- persistent activations ----
            pad = [qpool.tile([BLK, PADW, PADW], bf16, name=f"pad_{b}") for b in range(NBLK)]
            for b in range(NBLK):
                nc.gpsimd.memset(pad[b][:], 0.0)
            Q = [qpool.tile([128, N], bf16, name=f"Q_{r}") for r in range(NREG)]
            kpart = [qpool.tile([128, NT], f32, name=f"kpart_{r}") for r in range(NREG)]
            vks_sb = [qpool.tile([128, 128], bf16, name=f"vks_{r}") for r in range(NREG)]

            xbt = {p: [qpool.tile([128, N], bf16, name=f"x{p}_{k}")
                       for k in range(2)] for p in ("h", "l")}
            # nt-major order so pass1's first matmuls aren't waiting on the
            # tail of a p/k-major DMA stream
            for nt in range(NT):
                for p, dx in (("h", x_h), ("l", x_l)):
                    for k in range(2):
                        nc.sync.dma_start(
                            out=xbt[p][k][:, TN * nt:TN * (nt + 1)],
                            in_=dx[128 * k:128 * (k + 1), TN * nt:TN * (nt + 1)])

            def xb(p, k, nt):
                return xbt[p][k][:, TN * nt:TN * (nt + 1)]

            # (w, x) pairs for the compensated qkv product
            QKV_TERMS = (("h", "h"), ("h", "l"), ("l", "h"))

            # ================ pass 1: natural order -> padded image =========
            for b in range(NBLK):
                for nt in range(NT):
                    ps = ppool.tile([BLK, 8, WW], f32, name="ps1", tag="mm")
                    for k in range(2):
                        nc.tensor.matmul(
                            ps[:], w1h[k][:, BLK * b:BLK * (b + 1)],
                            xb("h", k, nt).rearrange("p (a c) -> p a c", c=WW),
                            start=(k == 0), stop=(k == 1))
                    nc.scalar.copy(out=pad[b][:, 2 + 8 * nt:10 + 8 * nt, 2:2 + WW], in_=ps[:])

            # ============ shared per-tile attention stage ====================
            def new_vkp(s_idx):
                return [ppool.tile([128, 128], f32, name=f"vkp_{s_idx}_{t}",
                                   tag="vk", bufs=2) for t in range(2)]

            def process_stage(s_idx, nt, ks, vs, vkp):
                """ks/vs: 2 bf16 [128,512] stage tiles (relu'd k / raw v)."""
                for t in range(2):
                    r = 2 * s_idx + t
                    nc.vector.reduce_sum(out=kpart[r][:, nt:nt + 1], in_=ks[t][:], axis=AX.X)
                for jj in range(4):
                    kT = spool.tile([128, 256], bf16, name="kT", tag="kT", bufs=2)
                    vT = spool.tile([128, 256], bf16, name="vT", tag="vT", bufs=2)
                    for t in range(2):
                        tp = ppool.tile([128, 128], bf16, name="tp", tag="mm")
                        nc.tensor.transpose(tp[:], ks[t][:, 128 * jj:128 * (jj + 1)], idt[:])
                        nc.scalar.copy(out=kT[:, 128 * t:128 * (t + 1)], in_=tp[:])
                        tp2 = ppool.tile([128, 128], bf16, name="tp2", tag="mm")
                        nc.tensor.transpose(tp2[:], vs[t][:, 128 * jj:128 * (jj + 1)], idt[:])
                        nc.vector.tensor_copy(out=vT[:, 128 * t:128 * (t + 1)], in_=tp2[:])
                    first = (nt == 0 and jj == 0)
                    last = (nt == NT - 1 and jj == 3)
                    for t in range(2):
                        nc.tensor.matmul(
                            vkp[t][:],
                            kT[:, 128 * t:128 * (t + 1)], vT[:, 128 * t:128 * (t + 1)],
                            start=first, stop=last)
                if nt == NT - 1:
                    for t in range(2):
                        nc.scalar.copy(out=vks_sb[2 * s_idx + t][:], in_=vkp[t][:])

            # ========== pass 2: separated order -> Q + id-scale k/v ==========
            vkp_id = new_vkp(0)
            for nt in range(NT):
                ks, vs = [None, None], [None, None]
                for j in range(6):
                    ps = ppool.tile([128, TN], f32, name="ps2", tag="mm")
                    for i, (pw_, px_) in enumerate(QKV_TERMS):
                        for k in range(2):
                            nc.tensor.matmul(ps[:], w2[pw_][k][:, 128 * j:128 * (j + 1)],
                                             xb(px_, k, nt),
                                             start=(i == 0 and k == 0),
                                             stop=(i == 2 and k == 1))
                    if j < 2:
                        nc.scalar.activation(out=Q[j][:, TN * nt:TN * (nt + 1)], in_=ps[:],
                                             func=AF.Relu, bias=bi2[j][:], scale=1.0)
                    elif j < 4:
                        t = j - 2
                        kst = spool.tile([128, TN], bf16, name="ks", tag=f"ks{t}", bufs=2)
                        nc.scalar.activation(out=kst[:], in_=ps[:], func=AF.Relu,
                                             bias=bi2[j][:], scale=1.0)
                        ks[t] = kst
                    else:
                        t = j - 4
                        vst = spool.tile([128, TN], bf16, name="vs", tag=f"vs{t}", bufs=2)
                        nc.vector.tensor_scalar(out=vst[:], in0=ps[:], scalar1=bi2[j][:],
                                                scalar2=None, op0=ALU.add)
                        vs[t] = vst
                process_stage(0, nt, ks, vs, vkp_id)

            # ================= fused conv scales =============================
            for s, taps, dfw, s_idx in ((3, TAPS3, dFW3, 1), (5, TAPS5, dFW5, 2)):
                vkp_s = new_vkp(s_idx)
                for h0 in range(0, NT, HALF):
                    stg = {}
                    for nth in range(HALF):
                        for t in range(2):
                            stg[("k", nth, t)] = spool.tile(
                                [128, TN], bf16, name="ks", tag=f"ks{t}", bufs=2)
                            stg[("v", nth, t)] = spool.tile(
                                [128, TN], bf16, name="vs", tag=f"vs{t}", bufs=2)
                    for b in range(NBLK):
                        # bufs=4: block b+1's accumulators must not wait on
                        # block b's PSUM eviction (was a 2.7us PE stall/block)
                        cps = [ppool.tile([BLK, 8, WW], f32, name="cp",
                                          tag="conv", bufs=4)
                               for _ in range(HALF)]
                        # fetch 2 taps per DMA trigger on the (otherwise idle)
                        # sync engine: per-(tap,block) gpsimd triggers used to
                        # occupy GpSimd ~660us, pacing the whole conv phase
                        npair = (len(taps) + 1) // 2
                        fwt2 = None
                        for ti, (dy, dx) in enumerate(taps):
                            if ti % 2 == 0:
                                fwt2 = wpool.tile([BLK, 2 * BLK], bf16,
                                                  name="fwt", tag="fw")
                                nc.sync.dma_start(
                                    out=fwt2[:], in_=dfw[b * npair + ti // 2])
                            fwt = fwt2[:, BLK * (ti % 2):BLK * (ti % 2 + 1)]
                            for nth in range(HALF):
                                nt = h0 + nth
                                nc.tensor.matmul(
                                    cps[nth][:], fwt,
                                    pad[b][:, 2 + 8 * nt + dy:10 + 8 * nt + dy,
                                           2 + dx:2 + dx + WW],
                                    start=(ti == 0), stop=(ti == len(taps) - 1))
                        qt, qr = (256 * s_idx + 32 * b) // 128, (32 * b) % 128
                        t2, r2 = b // 4, (32 * b) % 128
                        for nth in range(HALF):
                            nt = h0 + nth
                            cp = cps[nth]
                            nc.scalar.activation(
                                out=Q[qt][qr:qr + 32, TN * nt:TN * (nt + 1)],
                                in_=cp[0:32].rearrange("p a c -> p (a c)"),
                                func=AF.Relu, bias=bc[s][b][0:32, :], scale=1.0)
                            nc.scalar.activation(
                                out=stg[("k", nth, t2)][r2:r2 + 32, :],
                                in_=cp[32:64].rearrange("p a c -> p (a c)"),
                                func=AF.Relu, bias=bc[s][b][32:64, :], scale=1.0)
                            nc.vector.tensor_scalar(
                                out=stg[("v", nth, t2)][r2:r2 + 32, :],
                                in0=cp[64:96].rearrange("p a c -> p (a c)"),
                                scalar1=bc[s][b][64:96, :], scalar2=None, op0=ALU.add)
                    for nth in range(HALF):
                        process_stage(s_idx, h0 + nth,
                                      [stg[("k", nth, t)] for t in range(2)],
                                      [stg[("v", nth, t)] for t in range(2)], vkp_s)

            # ============== assemble apply weights from vk ===================
            apw2 = []
            denw = []
            for r in range(NREG):
                kf = qpool.tile([128, 1], f32, name=f"kfin_{r}")
                nc.vector.reduce_sum(out=kf[:], in_=kpart[r][:], axis=AX.X)
                vks = vks_sb[r]
                # den weights for quad-packed den matmuls: [128, 32] with this
                # region's two halves in columns 16*(r%2)..+16, zeros elsewhere
                dnw = qpool.tile([128, 32], bf16, name=f"denw_{r}")
                nc.gpsimd.memset(dnw[:], 0.0)
                # paired apply weights: both halves of the region in one
                # [128, 128] lhsT (one apply matmul per region per tile)
                aw2 = qpool.tile([128, 128], bf16, name=f"apw2_{r}")
                for half in range(2):
                    nc.vector.tensor_tensor(
                        out=aw2[:, 64 * half:64 * (half + 1)].rearrange(
                            "p (d h) -> p d h", h=8),
                        in0=vks[:, 64 * half:64 * (half + 1)].rearrange(
                            "p (h d) -> p d h", d=8),
                        in1=mp[half][:].rearrange("p (d h) -> p d h", h=8),
                        op=ALU.mult)
                    nc.vector.tensor_scalar(
                        out=dnw[:, 16 * (r % 2) + 8 * half:16 * (r % 2) + 8 * half + 8],
                        in0=md[half][:], scalar1=kf[:], scalar2=None, op0=ALU.mult)
                apw2.append(aw2)
                denw.append(dnw)

            # ================= apply + normalize + proj ======================
            for nt in range(NT):
                pjs = [ppool.tile([128, TN], f32, name=f"pj{m}", tag="conv", bufs=4)
                       for m in range(2)]
                # pass A: all 12 denominators -> one batched reciprocal
                # (per-group [8,512] reciprocal chains cost ~190us of vector
                # time and serialized against PE)
                den12 = spool.tile([96, TN], f32, name="den12", tag="den", bufs=2)
                for G in range(3):
                    # reuse the vk PSUM banks (dead after vk assembly): with
                    # tag "mm" the first den matmul of tile nt stalls ~3us on
                    # tile nt-1's aps/eps buffers draining
                    dps = ppool.tile([32, TN], f32, name="dps", tag="vk", bufs=2)
                    for rr in range(2):
                        r = 2 * G + rr
                        nc.tensor.matmul(dps[:], denw[r][:],
                                         Q[r][:, TN * nt:TN * (nt + 1)],
                                         start=(rr == 0), stop=(rr == 1))
                    nc.scalar.copy(out=den12[32 * G:32 * (G + 1), :], in_=dps[:])
                nc.vector.tensor_scalar(out=den12[:], in0=den12[:], scalar1=1e-15,
                                        scalar2=None, op0=ALU.add)
                rc12 = spool.tile([96, TN], f32, name="rc12", tag="rc", bufs=1)
                scr12 = spool.tile([96, TN], f32, name="scr12", tag="scr", bufs=1)
                nc.vector.reciprocal_approx_accurate(out=rc12[:], in_=den12[:],
                                                     scratch=scr12[:])
                # three base-0 tiles: matmul rhs must share base partition
                # with its lhsT (expw variants live at base 0)
                rcb32 = []
                for G in range(3):
                    rt = spool.tile([32, TN], bf16, name=f"rcb{G}", tag="rcb", bufs=3)
                    nc.scalar.copy(out=rt[:], in_=rc12[32 * G:32 * (G + 1), :])
                    rcb32.append(rt)
                # pass B: apply -> normalize -> proj, both halves of a region
                # paired into single [128,128]-lhsT matmuls
                for j in range(NREG):
                    aps = ppool.tile([128, TN], f32, name="aps", tag="mm")
                    nc.tensor.matmul(aps[:], apw2[j][:], Q[j][:, TN * nt:TN * (nt + 1)],
                                     start=True, stop=True)
                    eps = ppool.tile([128, TN], f32, name="eps", tag="mm")
                    nc.tensor.matmul(eps[:], expw[j % 2][:], rcb32[j // 2][:],
                                     start=True, stop=True)
                    exb = spool.tile([128, TN], f32, name="exb", tag="exb", bufs=2)
                    nc.scalar.copy(out=exb[:], in_=eps[:])
                    at = spool.tile([128, TN], bf16, name="at", tag="at", bufs=2)
                    nc.vector.tensor_tensor(out=at[:], in0=aps[:], in1=exb[:], op=ALU.mult)
                    for m in range(2):
                        nc.tensor.matmul(pjs[m][:], pwt[j][:, 128 * m:128 * (m + 1)],
                                         at[:], start=(j == 0), stop=(j == NREG - 1))
                for m in range(2):
                    ob = spool.tile([128, TN], f32, name="ob", tag="ob", bufs=2)
                    nc.vector.tensor_scalar(out=ob[:], in0=pjs[m][:], scalar1=pbt[m][:],
                                            scalar2=None, op0=ALU.add)
                    nc.sync.dma_start(
                        out=d_out[128 * m:128 * (m + 1), TN * nt:TN * (nt + 1)], in_=ob[:])
    return nc


def _get_nc():
    if "nc" not in _cache:
        nc = _build()
        nc.compile()
        _cache["nc"] = nc
    return _cache["nc"]


def _whash(inputs):
    h = hashlib.blake2b(digest_size=16)
    for name in ("qkv_w", "qkv_b", "dw3_w", "dw3_b", "pw3_w", "pw3_b",
                 "dw5_w", "dw5_b", "pw5_w", "pw5_b", "proj_w", "proj_b"):
        h.update(np.ascontiguousarray(np.asarray(inputs[name], np.float32)))
    return h.hexdigest()


def _feeds(inputs):
    import ml_dtypes

    def bf(a):
        return np.asarray(a, ml_dtypes.bfloat16)

    def split(a):
        hi = bf(a)
        lo = bf(np.asarray(a, np.float32) - np.asarray(hi, np.float32))
        return hi, lo

    key = _whash(inputs)
    if _cache.get("feeds_key") != key:
        d = _host_weights(inputs)
        w2h, w2l = split(d["w2t"])
        base = {
            "w1h": bf(d["w1t"]), "w2h": w2h, "w2l": w2l,
            "bi2": d["bi2"].astype(np.float32),
            "fw3": bf(d["fw3"]), "fw5": bf(d["fw5"]),
            "bc3": d["bc3"].astype(np.float32), "bc5": d["bc5"].astype(np.float32),
            "exp": bf(d["exp"]), "idt": bf(d["idt"]),
            "mp0": bf(d["mp0"]), "mp1": bf(d["mp1"]),
            "md0": bf(d["md0"]), "md1": bf(d["md1"]),
            "pw": bf(d["pw"]), "pb": d["pb"].astype(np.float32),
        }
        _cache["feeds_key"] = key
        _cache["feeds_base"] = base
    x = np.asarray(inputs["x"], np.float32).reshape(B, CIN, N)
    xh, xl = split(x)
    return _cache["feeds_base"], (xh, xl)


def _get_runner():
    """Build the jitted shard_map callable once; reuse across kernel() calls.

    Mirrors concourse.bass2jax.run_bass_via_pjrt but caches the jitted
    function (avoids re-lowering/re-compiling the XLA wrapper per call) and
    keeps the replicated weight operands device-resident.
    """
    if "runner" in _cache:
        return _cache["runner"]
    import jax
    import concourse.mybir as mybir
    from concourse import bass2jax
    from jax.experimental.shard_map import shard_map
    from jax.sharding import Mesh, PartitionSpec

    bass2jax.install_neuronx_cc_hook()
    nc = _get_nc()
    assert nc.dbg_addr is None or not nc.dbg_callbacks

    partition_name = (nc.partition_id_tensor.name
                      if nc.partition_id_tensor else None)
    in_names, out_names, out_avals = [], [], []
    for alloc in nc.m.functions[0].allocations:
        if not isinstance(alloc, mybir.MemoryLocationSet):
            continue
        name = alloc.memorylocations[0].name
        if alloc.kind == "ExternalInput":
            if name != partition_name:
                in_names.append(name)
        elif alloc.kind == "ExternalOutput":
            out_names.append(name)
            out_avals.append(jax.core.ShapedArray(
                tuple(alloc.tensor_shape), mybir.dt.np(alloc.dtype)))
    n_params = len(in_names)
    all_in = in_names + out_names + ([partition_name] if partition_name else [])

    def _body(*args):
        operands = list(args)
        if partition_name is not None:
            operands.append(bass2jax.partition_id_tensor())
        return tuple(bass2jax._bass_exec_p.bind(
            *operands,
            out_avals=tuple(out_avals),
            in_names=tuple(all_in),
            out_names=tuple(out_names),
            lowering_input_output_aliases=(),
            sim_require_finite=True,
            sim_require_nnan=True,
            nc=nc,
        ))

    devices = jax.devices()[:B]
    mesh = Mesh(np.asarray(devices), ("core",))
    sharded = jax.jit(
        shard_map(_body, mesh=mesh,
                  in_specs=(PartitionSpec("core"),) * (n_params + len(out_names)),
                  out_specs=(PartitionSpec("core"),) * len(out_names),
                  check_rep=False),
        keep_unused=True)
    _cache["runner"] = (sharded, in_names, out_names, out_avals, mesh)
    return _cache["runner"]


def kernel(**inputs):
    import jax
    from jax.sharding import NamedSharding, PartitionSpec

    base, (xh, xl) = _feeds(inputs)
    sharded, in_names, out_names, out_avals, mesh = _get_runner()

    sh = NamedSharding(mesh, PartitionSpec("core"))
    key = _cache["feeds_key"]
    if _cache.get("dev_key") != key:
        dev = {}
        for name in in_names:
            if name in ("xh", "xl"):
                continue
            a = np.asarray(base[name])
            rep = np.concatenate([a] * B, axis=0)
            dev[name] = jax.device_put(rep, sh)
        _cache["dev_key"] = key
        _cache["dev_weights"] = dev
    dev = _cache["dev_weights"]

    if "dev_zeros" not in _cache:
        _cache["dev_zeros"] = [
            jax.device_put(
                np.zeros((B * av.shape[0],) + tuple(av.shape[1:]), av.dtype), sh)
            for av in out_avals]

    xg = {"xh": xh, "xl": xl}
    args = []
    for name in in_names:
        if name in ("xh", "xl"):
            args.append(np.ascontiguousarray(xg[name].reshape(B * CIN, N)))
        else:
            args.append(dev[name])
    args.extend(_cache["dev_zeros"])

    out_arrs = sharded(*args)
    idx = out_names.index("out")
    out = np.asarray(out_arrs[idx]).reshape(B, CIN, HH, WW)
    return out.astype(np.float32)



# revision 87
# speedup vs baseline: 1.1407x; 1.0007x over previous
"""LiteMLA (EfficientViT multi-scale linear attention) Trainium2 Bass kernel.

Sharding: data-parallel over batch B=8 across 8 NeuronCores (1 image/core).
Per-core pipeline (matmul operands bf16, PSUM accumulation fp32):
  1. qkv = Wqkv @ x computed twice with host-permuted weights:
     pass1 (natural channel order) -> zero-padded SBUF image for conv taps,
     pass2 (q|k|v separated order) -> attention Q buffer + id-scale K/V stages.
  2. s3/s5: depthwise 3x3/5x5 + grouped 1x1 FUSED on host into per-tap
     block-diagonal [96,96] weights (4 head-groups per block); PE matmuls
     accumulate taps in PSUM reading shifted slices of the padded image.
  3. relu-linear attention: per spatial tile, relu(k)/v transposed on PE and
     reduced into per-16-head vk outer products (PSUM accumulated over all
     4096 positions); denominator comes from row-sums of relu(k).
  4. vk -> block-diagonal apply weights via host 0/1 masks (no tiny copies);
     out = vk @ relu(q) in dd-major layout so denominators are contiguous;
     normalize with reciprocal + PE broadcast-expand; proj uses a host-padded
     weight with zero rows on denominator positions.

All SBUF operand slices start at partition 0/32/64/96 (HW requirement).
"""

import hashlib
import sys

import numpy as np

sys.path.insert(0, "/opt/trn_rl_repo")

B, CIN, HH, WW = 8, 256, 64, 64
N = HH * WW            # 4096
HEADS = 32             # per scale
C3 = 768
NHEADS = 96
PADW = WW + 4          # 68
NT = 8                 # spatial tiles of 512 positions (8 image rows each)
TN = 512
HALF = 2               # nts processed per conv weight fetch
TAPS3 = [(dy, dx) for dy in (-1, 0, 1) for dx in (-1, 0, 1)]
TAPS5 = [(dy, dx) for dy in (-2, -1, 0, 1, 2) for dx in (-2, -1, 0, 1, 2)]
NBLK = 8               # conv channel blocks of 4 head-groups
BLK = 96
NREG = 6               # vk regions of 16 heads
NAPP = 12              # apply groups of 8 heads

_cache = {}


def _head_of(g12, i):
    return 16 * (g12 // 2) + 8 * (g12 % 2) + i


def _host_weights(inp):
    f32 = np.float32
    W = np.asarray(inp["qkv_w"], f32)[:, :, 0, 0]            # [768, 256]
    qkv_b = np.asarray(inp["qkv_b"], f32)
    pw = {3: np.asarray(inp["pw3_w"], f32)[:, :, 0, 0],
          5: np.asarray(inp["pw5_w"], f32)[:, :, 0, 0]}
    pwb = {3: np.asarray(inp["pw3_b"], f32), 5: np.asarray(inp["pw5_b"], f32)}
    dw = {3: np.asarray(inp["dw3_w"], f32)[:, 0],
          5: np.asarray(inp["dw5_w"], f32)[:, 0]}
    dwb = {3: np.asarray(inp["dw3_b"], f32), 5: np.asarray(inp["dw5_b"], f32)}
    proj_w = np.asarray(inp["proj_w"], f32)[:, :, 0, 0]      # [256, 768]
    proj_b = np.asarray(inp["proj_b"], f32)

    d = {}
    d["w1t"] = np.ascontiguousarray(W.T)                     # [256, 768]
    hh = np.repeat(np.arange(HEADS), 8)
    ee = np.tile(np.arange(8), HEADS)
    perm2 = np.concatenate([hh * 24 + ee, hh * 24 + 8 + ee, hh * 24 + 16 + ee])
    d["w2t"] = np.ascontiguousarray(W[perm2].T)
    d["bi2"] = qkv_b[perm2].reshape(768, 1)

    # fused conv weights: per tap, 8 blocks of 4 groups, [96in, 96out q|k|v]
    oo = np.arange(24)
    for s, taps in ((3, TAPS3), (5, TAPS5)):
        T = len(taps)
        M = pw[s].reshape(32, 24, 24)                        # [g, oo, i]
        dv = dw[s].reshape(32, 24, T)                        # [g, i, t]
        F = np.einsum("goi,git->tgio", M, dv)                # [t, g, i, oo]
        Ft = F.reshape(T, NBLK, 4, 24, 24)
        fw = np.zeros((T, NBLK, 4, 24, BLK), f32)
        bias24 = pwb[s].reshape(32, 24) + np.einsum(
            "goi,gi->go", M, dwb[s].reshape(32, 24))         # [g, oo]
        b24 = bias24.reshape(NBLK, 4, 24)
        fb = np.zeros((NBLK, 4, BLK), f32)
        for gl in range(4):
            m = (oo // 8) * 32 + gl * 8 + (oo % 8)           # [q32|k32|v32]
            fw[:, :, gl, :, m] = np.moveaxis(Ft[:, :, gl], -1, 0)
            fb[:, gl, m] = b24[:, gl]
        # pair-contiguous layout per block: [b, pair, 96, 2*96] so a
        # 2-tap weight fetch is one plain 2D DMA (odd tap counts get a
        # zero-padded, never-read second half in the last pair)
        P2 = (T + 1) // 2
        f4 = fw.reshape(T, NBLK, BLK, BLK)
        fwp = np.zeros((NBLK, P2, BLK, 2 * BLK), f32)
        for t in range(T):
            fwp[:, t // 2, :, (t % 2) * BLK:(t % 2 + 1) * BLK] = f4[t]
        d[f"fw{s}"] = fwp.reshape(NBLK * P2, BLK, 2 * BLK)
        d[f"bc{s}"] = fb.sum(axis=1).reshape(NBLK, BLK, 1)

    # expand lhsT: out row (dd,h) <- recip row h, two half-groups paired in
    # columns 0:64 / 64:128; 2 variants selecting which 16-row group of a
    # 32-row reciprocal block (SBUF partition offsets must be 32-multiples)
    E32 = np.zeros((2, 32, 128), f32)
    for v in range(2):
        for hf in range(2):
            for h in range(8):
                for dd in range(8):
                    E32[v, 16 * v + 8 * hf + h, 64 * hf + 8 * dd + h] = 1.0
    d["exp"] = E32
    d["idt"] = np.eye(128, dtype=f32)

    # masks for vk -> apply-weight assembly (dd-major cols)
    for half in range(2):
        mp = np.zeros((128, 64), f32)   # [(hp,e), (dd,h)]
        md = np.zeros((128, 8), f32)    # [(hp,e), h]
        for p in range(128):
            hp = p // 8
            for h in range(8):
                if hp == h + 8 * half:
                    md[p, h] = 1.0
                    for dd in range(8):
                        mp[p, 8 * dd + h] = 1.0
        d[f"mp{half}"] = mp
        d[f"md{half}"] = md

    # proj lhsT [6, 128, 256]: rows (half, dd, h) for the paired att layout
    g_ = np.arange(NAPP)[:, None]
    i_ = np.arange(8)[None, :]
    Hh = 16 * (g_ // 2) + 8 * (g_ % 2) + i_                  # [12, 8]
    dd_ = np.arange(8)
    cols = 8 * Hh[:, None, :] + dd_[None, :, None]           # [12, dd, i]
    PWm = proj_w.T[cols.reshape(NAPP, 64)]                   # [12, 64, 256]
    d["pw"] = PWm.reshape(6, 128, 256)
    d["pb"] = proj_b.reshape(256, 1)
    return d


def _build():
    import concourse.bass as bass
    import concourse.bacc as bacc_mod
    import concourse.mybir as mybir
    from concourse.tile import TileContext

    dt = mybir.dt
    f32, bf16 = dt.float32, dt.bfloat16
    f32r = dt.float32r
    AF = mybir.ActivationFunctionType
    ALU = mybir.AluOpType
    AX = mybir.AxisListType

    nc = bacc_mod.Bacc()
    # x and the qkv weights need ~fp32 operand precision: bf16 rounding of
    # these two operands alone produces ~0.2 rel err in the final output
    # (heavy cancellation downstream), and fp32r matmuls truncate operands
    # on real HW. So split both into bf16 hi+lo pairs and compute
    # W@x = Wh@xh + Wh@xl + Wl@xh (fp32 PSUM accumulation, wl@xl ~2^-18
    # negligible) at bf16 matmul speed.
    x_h = nc.dram_tensor("xh", [CIN, N], bf16, kind="ExternalInput")
    x_l = nc.dram_tensor("xl", [CIN, N], bf16, kind="ExternalInput")
    dW1h = nc.dram_tensor("w1h", [CIN, C3], bf16, kind="ExternalInput")
    dW2h = nc.dram_tensor("w2h", [CIN, C3], bf16, kind="ExternalInput")
    dW2l = nc.dram_tensor("w2l", [CIN, C3], bf16, kind="ExternalInput")
    dBI2 = nc.dram_tensor("bi2", [C3, 1], f32, kind="ExternalInput")
    dFW3 = nc.dram_tensor("fw3", [5 * NBLK, BLK, 2 * BLK], bf16, kind="ExternalInput")
    dFW5 = nc.dram_tensor("fw5", [13 * NBLK, BLK, 2 * BLK], bf16, kind="ExternalInput")
    dBC3 = nc.dram_tensor("bc3", [NBLK, BLK, 1], f32, kind="ExternalInput")
    dBC5 = nc.dram_tensor("bc5", [NBLK, BLK, 1], f32, kind="ExternalInput")
    dEXP = nc.dram_tensor("exp", [2, 32, 128], bf16, kind="ExternalInput")
    dIDT = nc.dram_tensor("idt", [128, 128], bf16, kind="ExternalInput")
    dMP = [nc.dram_tensor(f"mp{h}", [128, 64], bf16, kind="ExternalInput") for h in range(2)]
    dMD = [nc.dram_tensor(f"md{h}", [128, 8], bf16, kind="ExternalInput") for h in range(2)]
    dPW = nc.dram_tensor("pw", [NREG, 128, 256], bf16, kind="ExternalInput")
    dPB = nc.dram_tensor("pb", [256, 1], f32, kind="ExternalInput")
    d_out = nc.dram_tensor("out", [CIN, N], f32, kind="ExternalOutput")

    with TileContext(nc) as tc:
        with (
            tc.tile_pool(name="consts", bufs=1) as cpool,
            tc.tile_pool(name="persist", bufs=1) as qpool,
            tc.tile_pool(name="wstream", bufs=6) as wpool,
            tc.tile_pool(name="stage", bufs=2) as spool,
            tc.tile_pool(name="psum", bufs=2, space="PSUM") as ppool,
        ):
            # ---- constants ----
            # pass1 (conv-branch qkv) tolerates plain-bf16 operands (the conv
            # branch's contribution to the output error stays ~0.008 total),
            # so only pass2 carries the hi+lo compensated weights
            w1h = [cpool.tile([128, C3], bf16, name=f"w1h_{k}") for k in range(2)]
            for k in range(2):
                nc.sync.dma_start(out=w1h[k][:], in_=dW1h[128 * k:128 * (k + 1), :])
            w2 = {}
            for p, d2 in (("h", dW2h), ("l", dW2l)):
                w2[p] = [cpool.tile([128, C3], bf16, name=f"w2{p}_{k}")
                         for k in range(2)]
                for k in range(2):
                    nc.sync.dma_start(out=w2[p][k][:], in_=d2[128 * k:128 * (k + 1), :])
            bi2 = [cpool.tile([128, 1], f32, name=f"bi2_{j}") for j in range(6)]
            for j in range(6):
                nc.sync.dma_start(out=bi2[j][:], in_=dBI2[128 * j:128 * (j + 1), :])
            bc = {}
            for s, db in ((3, dBC3), (5, dBC5)):
                bc[s] = [cpool.tile([BLK, 1], f32, name=f"bc{s}_{b}") for b in range(NBLK)]
                for b in range(NBLK):
                    nc.sync.dma_start(out=bc[s][b][:], in_=db[b])
            expw = [cpool.tile([32, 128], bf16, name=f"expw_{v}") for v in range(2)]
            for v in range(2):
                nc.sync.dma_start(out=expw[v][:], in_=dEXP[v])
            idt = cpool.tile([128, 128], bf16, name="idt")
            nc.sync.dma_start(out=idt[:], in_=dIDT[:, :])
            mp = [cpool.tile([128, 64], bf16, name=f"mp_{h}") for h in range(2)]
            md = [cpool.tile([128, 8], bf16, name=f"md_{h}") for h in range(2)]
            for h in range(2):
                nc.sync.dma_start(out=mp[h][:], in_=dMP[h][:, :])
                nc.sync.dma_start(out=md[h][:], in_=dMD[h][:, :])
            pwt = [cpool.tile([128, 256], bf16, name=f"pwt_{g}") for g in range(NREG)]
            for g in range(NREG):
                nc.sync.dma_start(out=pwt[g][:], in_=dPW[g])
            pbt = [cpool.tile([128, 1], f32, name=f"pbt_{m}") for m in range(2)]
            for m in range(2):
                nc.sync.dma_start(out=pbt[m][:], in_=dPB[128 * m:128 * (m + 1), :])

            # ---- persistent activations ----
            pad = [qpool.tile([BLK, PADW, PADW], bf16, name=f"pad_{b}") for b in range(NBLK)]
            for b in range(NBLK):
                nc.gpsimd.memset(pad[b][:], 0.0)
            Q = [qpool.tile([128, N], bf16, name=f"Q_{r}") for r in range(NREG)]
            kpart = [qpool.tile([128, NT], f32, name=f"kpart_{r}") for r in range(NREG)]
            vks_sb = [qpool.tile([128, 128], bf16, name=f"vks_{r}") for r in range(NREG)]

            xbt = {p: [qpool.tile([128, N], bf16, name=f"x{p}_{k}")
                       for k in range(2)] for p in ("h", "l")}
            # nt-major order so pass1's first matmuls aren't waiting on the
            # tail of a p/k-major DMA stream
            for nt in range(NT):
                for p, dx in (("h", x_h), ("l", x_l)):
                    for k in range(2):
                        nc.sync.dma_start(
                            out=xbt[p][k][:, TN * nt:TN * (nt + 1)],
                            in_=dx[128 * k:128 * (k + 1), TN * nt:TN * (nt + 1)])

            def xb(p, k, nt):
                return xbt[p][k][:, TN * nt:TN * (nt + 1)]

            # (w, x) pairs for the compensated qkv product
            QKV_TERMS = (("h", "h"), ("h", "l"), ("l", "h"))

            # ================ pass 1: natural order -> padded image =========
            for b in range(NBLK):
                for nt in range(NT):
                    ps = ppool.tile([BLK, 8, WW], f32, name="ps1", tag="mm")
                    for k in range(2):
                        nc.tensor.matmul(
                            ps[:], w1h[k][:, BLK * b:BLK * (b + 1)],
                            xb("h", k, nt).rearrange("p (a c) -> p a c", c=WW),
                            start=(k == 0), stop=(k == 1))
                    nc.scalar.copy(out=pad[b][:, 2 + 8 * nt:10 + 8 * nt, 2:2 + WW], in_=ps[:])

            # ============ shared per-tile attention stage ====================
            def new_vkp(s_idx):
                return [ppool.tile([128, 128], f32, name=f"vkp_{s_idx}_{t}",
                                   tag="vk", bufs=2) for t in range(2)]

            def process_stage(s_idx, nt, ks, vs, vkp):
                """ks/vs: 2 bf16 [128,512] stage tiles (relu'd k / raw v)."""
                for t in range(2):
                    r = 2 * s_idx + t
                    nc.vector.reduce_sum(out=kpart[r][:, nt:nt + 1], in_=ks[t][:], axis=AX.X)
                for jj in range(4):
                    kT = spool.tile([128, 256], bf16, name="kT", tag="kT", bufs=2)
                    vT = spool.tile([128, 256], bf16, name="vT", tag="vT", bufs=2)
                    for t in range(2):
                        tp = ppool.tile([128, 128], bf16, name="tp", tag="mm")
                        nc.tensor.transpose(tp[:], ks[t][:, 128 * jj:128 * (jj + 1)], idt[:])
                        # vector copy (~215ns) not scalar (~500ns): the tp
                        # eviction latency gates the shared mm-tag PSUM
                        # rotation that qkv/conv matmuls also cycle through
                        nc.vector.tensor_copy(out=kT[:, 128 * t:128 * (t + 1)], in_=tp[:])
                        tp2 = ppool.tile([128, 128], bf16, name="tp2", tag="mm")
                        nc.tensor.transpose(tp2[:], vs[t][:, 128 * jj:128 * (jj + 1)], idt[:])
                        nc.vector.tensor_copy(out=vT[:, 128 * t:128 * (t + 1)], in_=tp2[:])
                    first = (nt == 0 and jj == 0)
                    last = (nt == NT - 1 and jj == 3)
                    for t in range(2):
                        nc.tensor.matmul(
                            vkp[t][:],
                            kT[:, 128 * t:128 * (t + 1)], vT[:, 128 * t:128 * (t + 1)],
                            start=first, stop=last)
                if nt == NT - 1:
                    for t in range(2):
                        nc.scalar.copy(out=vks_sb[2 * s_idx + t][:], in_=vkp[t][:])

            # ========== pass 2: separated order -> Q + id-scale k/v ==========
            vkp_id = new_vkp(0)
            for nt in range(NT):
                ks, vs = [None, None], [None, None]
                for j in range(6):
                    ps = ppool.tile([128, TN], f32, name="ps2", tag="mm")
                    for i, (pw_, px_) in enumerate(QKV_TERMS):
                        for k in range(2):
                            nc.tensor.matmul(ps[:], w2[pw_][k][:, 128 * j:128 * (j + 1)],
                                             xb(px_, k, nt),
                                             start=(i == 0 and k == 0),
                                             stop=(i == 2 and k == 1))
                    if j < 2:
                        nc.scalar.activation(out=Q[j][:, TN * nt:TN * (nt + 1)], in_=ps[:],
                                             func=AF.Relu, bias=bi2[j][:], scale=1.0)
                    elif j < 4:
                        t = j - 2
                        kst = spool.tile([128, TN], bf16, name="ks", tag=f"ks{t}", bufs=2)
                        nc.scalar.activation(out=kst[:], in_=ps[:], func=AF.Relu,
                                             bias=bi2[j][:], scale=1.0)
                        ks[t] = kst
                    else:
                        t = j - 4
                        vst = spool.tile([128, TN], bf16, name="vs", tag=f"vs{t}", bufs=2)
                        nc.vector.tensor_scalar(out=vst[:], in0=ps[:], scalar1=bi2[j][:],
                                                scalar2=None, op0=ALU.add)
                        vs[t] = vst
                process_stage(0, nt, ks, vs, vkp_id)

            # ================= fused conv scales =============================
            for s, taps, dfw, s_idx in ((3, TAPS3, dFW3, 1), (5, TAPS5, dFW5, 2)):
                vkp_s = new_vkp(s_idx)
                for h0 in range(0, NT, HALF):
                    stg = {}
                    for nth in range(HALF):
                        for t in range(2):
                            stg[("k", nth, t)] = spool.tile(
                                [128, TN], bf16, name="ks", tag=f"ks{t}", bufs=2)
                            stg[("v", nth, t)] = spool.tile(
                                [128, TN], bf16, name="vs", tag=f"vs{t}", bufs=2)
                    for b in range(NBLK):
                        # bufs=4: block b+1's accumulators must not wait on
                        # block b's PSUM eviction (was a 2.7us PE stall/block)
                        cps = [ppool.tile([BLK, 8, WW], f32, name="cp",
                                          tag="conv", bufs=4)
                               for _ in range(HALF)]
                        # fetch 2 taps per DMA trigger on the (otherwise idle)
                        # sync engine: per-(tap,block) gpsimd triggers used to
                        # occupy GpSimd ~660us, pacing the whole conv phase
                        npair = (len(taps) + 1) // 2
                        fwt2 = None
                        for ti, (dy, dx) in enumerate(taps):
                            if ti % 2 == 0:
                                fwt2 = wpool.tile([BLK, 2 * BLK], bf16,
                                                  name="fwt", tag="fw")
                                nc.sync.dma_start(
                                    out=fwt2[:], in_=dfw[b * npair + ti // 2])
                            fwt = fwt2[:, BLK * (ti % 2):BLK * (ti % 2 + 1)]
                            for nth in range(HALF):
                                nt = h0 + nth
                                nc.tensor.matmul(
                                    cps[nth][:], fwt,
                                    pad[b][:, 2 + 8 * nt + dy:10 + 8 * nt + dy,
                                           2 + dx:2 + dx + WW],
                                    start=(ti == 0), stop=(ti == len(taps) - 1))
                        qt, qr = (256 * s_idx + 32 * b) // 128, (32 * b) % 128
                        t2, r2 = b // 4, (32 * b) % 128
                        for nth in range(HALF):
                            nt = h0 + nth
                            cp = cps[nth]
                            nc.scalar.activation(
                                out=Q[qt][qr:qr + 32, TN * nt:TN * (nt + 1)],
                                in_=cp[0:32].rearrange("p a c -> p (a c)"),
                                func=AF.Relu, bias=bc[s][b][0:32, :], scale=1.0)
                            nc.scalar.activation(
                                out=stg[("k", nth, t2)][r2:r2 + 32, :],
                                in_=cp[32:64].rearrange("p a c -> p (a c)"),
                                func=AF.Relu, bias=bc[s][b][32:64, :], scale=1.0)
                            nc.vector.tensor_scalar(
                                out=stg[("v", nth, t2)][r2:r2 + 32, :],
                                in0=cp[64:96].rearrange("p a c -> p (a c)"),
                                scalar1=bc[s][b][64:96, :], scalar2=None, op0=ALU.add)
                    for nth in range(HALF):
                        process_stage(s_idx, h0 + nth,
                                      [stg[("k", nth, t)] for t in range(2)],
                                      [stg[("v", nth, t)] for t in range(2)], vkp_s)

            # ============== assemble apply weights from vk ===================
            apw2 = []
            denw = []
            for r in range(NREG):
                kf = qpool.tile([128, 1], f32, name=f"kfin_{r}")
                nc.vector.reduce_sum(out=kf[:], in_=kpart[r][:], axis=AX.X)
                vks = vks_sb[r]
                # den weights for quad-packed den matmuls: [128, 32] with this
                # region's two halves in columns 16*(r%2)..+16, zeros elsewhere
                dnw = qpool.tile([128, 32], bf16, name=f"denw_{r}")
                nc.gpsimd.memset(dnw[:], 0.0)
                # paired apply weights: both halves of the region in one
                # [128, 128] lhsT (one apply matmul per region per tile)
                aw2 = qpool.tile([128, 128], bf16, name=f"apw2_{r}")
                for half in range(2):
                    nc.vector.tensor_tensor(
                        out=aw2[:, 64 * half:64 * (half + 1)].rearrange(
                            "p (d h) -> p d h", h=8),
                        in0=vks[:, 64 * half:64 * (half + 1)].rearrange(
                            "p (h d) -> p d h", d=8),
                        in1=mp[half][:].rearrange("p (d h) -> p d h", h=8),
                        op=ALU.mult)
                    nc.vector.tensor_scalar(
                        out=dnw[:, 16 * (r % 2) + 8 * half:16 * (r % 2) + 8 * half + 8],
                        in0=md[half][:], scalar1=kf[:], scalar2=None, op0=ALU.mult)
                apw2.append(aw2)
                denw.append(dnw)

            # ================= apply + normalize + proj ======================
            for nt in range(NT):
                pjs = [ppool.tile([128, TN], f32, name=f"pj{m}", tag="conv", bufs=4)
                       for m in range(2)]
                # pass A: all 12 denominators -> one batched reciprocal
                # (per-group [8,512] reciprocal chains cost ~190us of vector
                # time and serialized against PE)
                den12 = spool.tile([96, TN], f32, name="den12", tag="den", bufs=2)
                for G in range(3):
                    # reuse the vk PSUM banks (dead after vk assembly): with
                    # tag "mm" the first den matmul of tile nt stalls ~3us on
                    # tile nt-1's aps/eps buffers draining
                    dps = ppool.tile([32, TN], f32, name="dps", tag="vk", bufs=2)
                    for rr in range(2):
                        r = 2 * G + rr
                        nc.tensor.matmul(dps[:], denw[r][:],
                                         Q[r][:, TN * nt:TN * (nt + 1)],
                                         start=(rr == 0), stop=(rr == 1))
                    nc.scalar.copy(out=den12[32 * G:32 * (G + 1), :], in_=dps[:])
                nc.vector.tensor_scalar(out=den12[:], in0=den12[:], scalar1=1e-15,
                                        scalar2=None, op0=ALU.add)
                rc12 = spool.tile([96, TN], f32, name="rc12", tag="rc", bufs=1)
                scr12 = spool.tile([96, TN], f32, name="scr12", tag="scr", bufs=1)
                nc.vector.reciprocal_approx_accurate(out=rc12[:], in_=den12[:],
                                                     scratch=scr12[:])
                # three base-0 tiles: matmul rhs must share base partition
                # with its lhsT (expw variants live at base 0)
                rcb32 = []
                for G in range(3):
                    rt = spool.tile([32, TN], bf16, name=f"rcb{G}", tag="rcb", bufs=3)
                    nc.scalar.copy(out=rt[:], in_=rc12[32 * G:32 * (G + 1), :])
                    rcb32.append(rt)
                # pass B: apply -> normalize -> proj, both halves of a region
                # paired into single [128,128]-lhsT matmuls
                for j in range(NREG):
                    aps = ppool.tile([128, TN], f32, name="aps", tag="mm")
                    nc.tensor.matmul(aps[:], apw2[j][:], Q[j][:, TN * nt:TN * (nt + 1)],
                                     start=True, stop=True)
                    eps = ppool.tile([128, TN], f32, name="eps", tag="mm")
                    nc.tensor.matmul(eps[:], expw[j % 2][:], rcb32[j // 2][:],
                                     start=True, stop=True)
                    exb = spool.tile([128, TN], f32, name="exb", tag="exb", bufs=2)
                    nc.scalar.copy(out=exb[:], in_=eps[:])
                    at = spool.tile([128, TN], bf16, name="at", tag="at", bufs=2)
                    nc.vector.tensor_tensor(out=at[:], in0=aps[:], in1=exb[:], op=ALU.mult)
                    for m in range(2):
                        nc.tensor.matmul(pjs[m][:], pwt[j][:, 128 * m:128 * (m + 1)],
                                         at[:], start=(j == 0), stop=(j == NREG - 1))
                for m in range(2):
                    ob = spool.tile([128, TN], f32, name="ob", tag="ob", bufs=2)
                    nc.vector.tensor_scalar(out=ob[:], in0=pjs[m][:], scalar1=pbt[m][:],
                                            scalar2=None, op0=ALU.add)
                    nc.sync.dma_start(
                        out=d_out[128 * m:128 * (m + 1), TN * nt:TN * (nt + 1)], in_=ob[:])
    return nc


def _get_nc():
    if "nc" not in _cache:
        nc = _build()
        nc.compile()
        _cache["nc"] = nc
    return _cache["nc"]


def _whash(inputs):
    h = hashlib.blake2b(digest_size=16)
    for name in ("qkv_w", "qkv_b", "dw3_w", "dw3_b", "pw3_w", "pw3_b",
                 "dw5_w", "dw5_b", "pw5_w", "pw5_b", "proj_w", "proj_b"):
        h.update(np.ascontiguousarray(np.asarray(inputs[name], np.float32)))
    return h.hexdigest()


def _feeds(inputs):
    import ml_dtypes

    def bf(a):
        return np.asarray(a, ml_dtypes.bfloat16)

    def split(a):
        hi = bf(a)
        lo = bf(np.asarray(a, np.float32) - np.asarray(hi, np.float32))
        return hi, lo

    key = _whash(inputs)
    if _cache.get("feeds_key") != key:
        d = _host_weights(inputs)
        w2h, w2l = split(d["w2t"])
        base = {
            "w1h": bf(d["w1t"]), "w2h": w2h, "w2l": w2l,
            "bi2": d["bi2"].astype(np.float32),
            "fw3": bf(d["fw3"]), "fw5": bf(d["fw5"]),
            "bc3": d["bc3"].astype(np.float32), "bc5": d["bc5"].astype(np.float32),
            "exp": bf(d["exp"]), "idt": bf(d["idt"]),
            "mp0": bf(d["mp0"]), "mp1": bf(d["mp1"]),
            "md0": bf(d["md0"]), "md1": bf(d["md1"]),
            "pw": bf(d["pw"]), "pb": d["pb"].astype(np.float32),
        }
        _cache["feeds_key"] = key
        _cache["feeds_base"] = base
    x = np.asarray(inputs["x"], np.float32).reshape(B, CIN, N)
    xh, xl = split(x)
    return _cache["feeds_base"], (xh, xl)


def _get_runner():
    """Build the jitted shard_map callable once; reuse across kernel() calls.

    Mirrors concourse.bass2jax.run_bass_via_pjrt but caches the jitted
    function (avoids re-lowering/re-compiling the XLA wrapper per call) and
    keeps the replicated weight operands device-resident.
    """
    if "runner" in _cache:
        return _cache["runner"]
    import jax
    import concourse.mybir as mybir
    from concourse import bass2jax
    from jax.experimental.shard_map import shard_map
    from jax.sharding import Mesh, PartitionSpec

    bass2jax.install_neuronx_cc_hook()
    nc = _get_nc()
    assert nc.dbg_addr is None or not nc.dbg_callbacks

    partition_name = (nc.partition_id_tensor.name
                      if nc.partition_id_tensor else None)
    in_names, out_names, out_avals = [], [], []
    for alloc in nc.m.functions[0].allocations:
        if not isinstance(alloc, mybir.MemoryLocationSet):
            continue
        name = alloc.memorylocations[0].name
        if alloc.kind == "ExternalInput":
            if name != partition_name:
                in_names.append(name)
        elif alloc.kind == "ExternalOutput":
            out_names.append(name)
            out_avals.append(jax.core.ShapedArray(
                tuple(alloc.tensor_shape), mybir.dt.np(alloc.dtype)))
    n_params = len(in_names)
    all_in = in_names + out_names + ([partition_name] if partition_name else [])

    def _body(*args):
        operands = list(args)
        if partition_name is not None:
            operands.append(bass2jax.partition_id_tensor())
        return tuple(bass2jax._bass_exec_p.bind(
            *operands,
            out_avals=tuple(out_avals),
            in_names=tuple(all_in),
            out_names=tuple(out_names),
            lowering_input_output_aliases=(),
            sim_require_finite=True,
            sim_require_nnan=True,
            nc=nc,
        ))

    devices = jax.devices()[:B]
    mesh = Mesh(np.asarray(devices), ("core",))
    sharded = jax.jit(
        shard_map(_body, mesh=mesh,
                  in_specs=(PartitionSpec("core"),) * (n_params + len(out_names)),
                  out_specs=(PartitionSpec("core"),) * len(out_names),
                  check_rep=False),
        keep_unused=True)
    _cache["runner"] = (sharded, in_names, out_names, out_avals, mesh)
    return _cache["runner"]


def kernel(**inputs):
    import jax
    from jax.sharding import NamedSharding, PartitionSpec

    base, (xh, xl) = _feeds(inputs)
    sharded, in_names, out_names, out_avals, mesh = _get_runner()

    sh = NamedSharding(mesh, PartitionSpec("core"))
    key = _cache["feeds_key"]
    if _cache.get("dev_key") != key:
        dev = {}
        for name in in_names:
            if name in ("xh", "xl"):
                continue
            a = np.asarray(base[name])
            rep = np.concatenate([a] * B, axis=0)
            dev[name] = jax.device_put(rep, sh)
        _cache["dev_key"] = key
        _cache["dev_weights"] = dev
    dev = _cache["dev_weights"]

    if "dev_zeros" not in _cache:
        _cache["dev_zeros"] = [
            jax.device_put(
                np.zeros((B * av.shape[0],) + tuple(av.shape[1:]), av.dtype), sh)
            for av in out_avals]

    xg = {"xh": xh, "xl": xl}
    args = []
    for name in in_names:
        if name in ("xh", "xl"):
            args.append(np.ascontiguousarray(xg[name].reshape(B * CIN, N)))
        else:
            args.append(dev[name])
    args.extend(_cache["dev_zeros"])

    out_arrs = sharded(*args)
    idx = out_names.index("out")
    out = np.asarray(out_arrs[idx]).reshape(B, CIN, HH, WW)
    return out.astype(np.float32)



# revision 88
# speedup vs baseline: 1.1580x; 1.0151x over previous
"""LiteMLA (EfficientViT multi-scale linear attention) Trainium2 Bass kernel.

Sharding: data-parallel over batch B=8 across 8 NeuronCores (1 image/core).
Per-core pipeline (matmul operands bf16, PSUM accumulation fp32):
  1. qkv = Wqkv @ x computed twice with host-permuted weights:
     pass1 (natural channel order) -> zero-padded SBUF image for conv taps,
     pass2 (q|k|v separated order) -> attention Q buffer + id-scale K/V stages.
  2. s3/s5: depthwise 3x3/5x5 + grouped 1x1 FUSED on host into per-tap
     block-diagonal [96,96] weights (4 head-groups per block); PE matmuls
     accumulate taps in PSUM reading shifted slices of the padded image.
  3. relu-linear attention: per spatial tile, relu(k)/v transposed on PE and
     reduced into per-16-head vk outer products (PSUM accumulated over all
     4096 positions); denominator comes from row-sums of relu(k).
  4. vk -> block-diagonal apply weights via host 0/1 masks (no tiny copies);
     out = vk @ relu(q) in dd-major layout so denominators are contiguous;
     normalize with reciprocal + PE broadcast-expand; proj uses a host-padded
     weight with zero rows on denominator positions.

All SBUF operand slices start at partition 0/32/64/96 (HW requirement).
"""

import hashlib
import sys

import numpy as np

sys.path.insert(0, "/opt/trn_rl_repo")

B, CIN, HH, WW = 8, 256, 64, 64
N = HH * WW            # 4096
HEADS = 32             # per scale
C3 = 768
NHEADS = 96
PADW = WW + 4          # 68
NT = 8                 # spatial tiles of 512 positions (8 image rows each)
TN = 512
HALF = 2               # nts processed per conv weight fetch
TAPS3 = [(dy, dx) for dy in (-1, 0, 1) for dx in (-1, 0, 1)]
TAPS5 = [(dy, dx) for dy in (-2, -1, 0, 1, 2) for dx in (-2, -1, 0, 1, 2)]
NBLK = 8               # conv channel blocks of 4 head-groups
BLK = 96
NREG = 6               # vk regions of 16 heads
NAPP = 12              # apply groups of 8 heads

_cache = {}


def _head_of(g12, i):
    return 16 * (g12 // 2) + 8 * (g12 % 2) + i


def _host_weights(inp):
    f32 = np.float32
    W = np.asarray(inp["qkv_w"], f32)[:, :, 0, 0]            # [768, 256]
    qkv_b = np.asarray(inp["qkv_b"], f32)
    pw = {3: np.asarray(inp["pw3_w"], f32)[:, :, 0, 0],
          5: np.asarray(inp["pw5_w"], f32)[:, :, 0, 0]}
    pwb = {3: np.asarray(inp["pw3_b"], f32), 5: np.asarray(inp["pw5_b"], f32)}
    dw = {3: np.asarray(inp["dw3_w"], f32)[:, 0],
          5: np.asarray(inp["dw5_w"], f32)[:, 0]}
    dwb = {3: np.asarray(inp["dw3_b"], f32), 5: np.asarray(inp["dw5_b"], f32)}
    proj_w = np.asarray(inp["proj_w"], f32)[:, :, 0, 0]      # [256, 768]
    proj_b = np.asarray(inp["proj_b"], f32)

    d = {}
    d["w1t"] = np.ascontiguousarray(W.T)                     # [256, 768]
    hh = np.repeat(np.arange(HEADS), 8)
    ee = np.tile(np.arange(8), HEADS)
    perm2 = np.concatenate([hh * 24 + ee, hh * 24 + 8 + ee, hh * 24 + 16 + ee])
    d["w2t"] = np.ascontiguousarray(W[perm2].T)
    d["bi2"] = qkv_b[perm2].reshape(768, 1)

    # fused conv weights: per tap, 8 blocks of 4 groups, [96in, 96out q|k|v]
    oo = np.arange(24)
    for s, taps in ((3, TAPS3), (5, TAPS5)):
        T = len(taps)
        M = pw[s].reshape(32, 24, 24)                        # [g, oo, i]
        dv = dw[s].reshape(32, 24, T)                        # [g, i, t]
        F = np.einsum("goi,git->tgio", M, dv)                # [t, g, i, oo]
        Ft = F.reshape(T, NBLK, 4, 24, 24)
        fw = np.zeros((T, NBLK, 4, 24, BLK), f32)
        bias24 = pwb[s].reshape(32, 24) + np.einsum(
            "goi,gi->go", M, dwb[s].reshape(32, 24))         # [g, oo]
        b24 = bias24.reshape(NBLK, 4, 24)
        fb = np.zeros((NBLK, 4, BLK), f32)
        for gl in range(4):
            m = (oo // 8) * 32 + gl * 8 + (oo % 8)           # [q32|k32|v32]
            fw[:, :, gl, :, m] = np.moveaxis(Ft[:, :, gl], -1, 0)
            fb[:, gl, m] = b24[:, gl]
        # pair-contiguous layout per block: [b, pair, 96, 2*96] so a
        # 2-tap weight fetch is one plain 2D DMA (odd tap counts get a
        # zero-padded, never-read second half in the last pair)
        P2 = (T + 1) // 2
        f4 = fw.reshape(T, NBLK, BLK, BLK)
        fwp = np.zeros((NBLK, P2, BLK, 2 * BLK), f32)
        for t in range(T):
            fwp[:, t // 2, :, (t % 2) * BLK:(t % 2 + 1) * BLK] = f4[t]
        d[f"fw{s}"] = fwp.reshape(NBLK * P2, BLK, 2 * BLK)
        d[f"bc{s}"] = fb.sum(axis=1).reshape(NBLK, BLK, 1)

    # expand lhsT: out row (dd,h) <- recip row h, two half-groups paired in
    # columns 0:64 / 64:128; 2 variants selecting which 16-row group of a
    # 32-row reciprocal block (SBUF partition offsets must be 32-multiples)
    E32 = np.zeros((2, 32, 128), f32)
    for v in range(2):
        for hf in range(2):
            for h in range(8):
                for dd in range(8):
                    E32[v, 16 * v + 8 * hf + h, 64 * hf + 8 * dd + h] = 1.0
    d["exp"] = E32
    d["idt"] = np.eye(128, dtype=f32)

    # masks for vk -> apply-weight assembly (dd-major cols)
    for half in range(2):
        mp = np.zeros((128, 64), f32)   # [(hp,e), (dd,h)]
        md = np.zeros((128, 8), f32)    # [(hp,e), h]
        for p in range(128):
            hp = p // 8
            for h in range(8):
                if hp == h + 8 * half:
                    md[p, h] = 1.0
                    for dd in range(8):
                        mp[p, 8 * dd + h] = 1.0
        d[f"mp{half}"] = mp
        d[f"md{half}"] = md

    # proj lhsT [6, 128, 256]: rows (half, dd, h) for the paired att layout
    g_ = np.arange(NAPP)[:, None]
    i_ = np.arange(8)[None, :]
    Hh = 16 * (g_ // 2) + 8 * (g_ % 2) + i_                  # [12, 8]
    dd_ = np.arange(8)
    cols = 8 * Hh[:, None, :] + dd_[None, :, None]           # [12, dd, i]
    PWm = proj_w.T[cols.reshape(NAPP, 64)]                   # [12, 64, 256]
    d["pw"] = PWm.reshape(6, 128, 256)
    d["pb"] = proj_b.reshape(256, 1)
    return d


def _build():
    import concourse.bass as bass
    import concourse.bacc as bacc_mod
    import concourse.mybir as mybir
    from concourse.tile import TileContext

    dt = mybir.dt
    f32, bf16 = dt.float32, dt.bfloat16
    f32r = dt.float32r
    AF = mybir.ActivationFunctionType
    ALU = mybir.AluOpType
    AX = mybir.AxisListType

    nc = bacc_mod.Bacc()
    # x and the qkv weights need ~fp32 operand precision: bf16 rounding of
    # these two operands alone produces ~0.2 rel err in the final output
    # (heavy cancellation downstream), and fp32r matmuls truncate operands
    # on real HW. So split both into bf16 hi+lo pairs and compute
    # W@x = Wh@xh + Wh@xl + Wl@xh (fp32 PSUM accumulation, wl@xl ~2^-18
    # negligible) at bf16 matmul speed.
    x_h = nc.dram_tensor("xh", [CIN, N], bf16, kind="ExternalInput")
    x_l = nc.dram_tensor("xl", [CIN, N], bf16, kind="ExternalInput")
    dW1h = nc.dram_tensor("w1h", [CIN, C3], bf16, kind="ExternalInput")
    dW2h = nc.dram_tensor("w2h", [CIN, C3], bf16, kind="ExternalInput")
    dW2l = nc.dram_tensor("w2l", [CIN, C3], bf16, kind="ExternalInput")
    dBI2 = nc.dram_tensor("bi2", [C3, 1], f32, kind="ExternalInput")
    dFW3 = nc.dram_tensor("fw3", [5 * NBLK, BLK, 2 * BLK], bf16, kind="ExternalInput")
    dFW5 = nc.dram_tensor("fw5", [13 * NBLK, BLK, 2 * BLK], bf16, kind="ExternalInput")
    dBC3 = nc.dram_tensor("bc3", [NBLK, BLK, 1], f32, kind="ExternalInput")
    dBC5 = nc.dram_tensor("bc5", [NBLK, BLK, 1], f32, kind="ExternalInput")
    dEXP = nc.dram_tensor("exp", [2, 32, 128], bf16, kind="ExternalInput")
    dIDT = nc.dram_tensor("idt", [128, 128], bf16, kind="ExternalInput")
    dMP = [nc.dram_tensor(f"mp{h}", [128, 64], bf16, kind="ExternalInput") for h in range(2)]
    dMD = [nc.dram_tensor(f"md{h}", [128, 8], bf16, kind="ExternalInput") for h in range(2)]
    dPW = nc.dram_tensor("pw", [NREG, 128, 256], bf16, kind="ExternalInput")
    dPB = nc.dram_tensor("pb", [256, 1], f32, kind="ExternalInput")
    d_out = nc.dram_tensor("out", [CIN, N], f32, kind="ExternalOutput")

    with TileContext(nc) as tc:
        with (
            tc.tile_pool(name="consts", bufs=1) as cpool,
            tc.tile_pool(name="persist", bufs=1) as qpool,
            tc.tile_pool(name="wstream", bufs=6) as wpool,
            tc.tile_pool(name="stage", bufs=2) as spool,
            tc.tile_pool(name="psum", bufs=2, space="PSUM") as ppool,
        ):
            # ---- constants ----
            # pass1 (conv-branch qkv) tolerates plain-bf16 operands (the conv
            # branch's contribution to the output error stays ~0.008 total),
            # so only pass2 carries the hi+lo compensated weights
            w1h = [cpool.tile([128, C3], bf16, name=f"w1h_{k}") for k in range(2)]
            for k in range(2):
                nc.sync.dma_start(out=w1h[k][:], in_=dW1h[128 * k:128 * (k + 1), :])
            w2 = {}
            for p, d2 in (("h", dW2h), ("l", dW2l)):
                w2[p] = [cpool.tile([128, C3], bf16, name=f"w2{p}_{k}")
                         for k in range(2)]
                for k in range(2):
                    nc.sync.dma_start(out=w2[p][k][:], in_=d2[128 * k:128 * (k + 1), :])
            bi2 = [cpool.tile([128, 1], f32, name=f"bi2_{j}") for j in range(6)]
            for j in range(6):
                nc.sync.dma_start(out=bi2[j][:], in_=dBI2[128 * j:128 * (j + 1), :])
            bc = {}
            for s, db in ((3, dBC3), (5, dBC5)):
                bc[s] = [cpool.tile([BLK, 1], f32, name=f"bc{s}_{b}") for b in range(NBLK)]
                for b in range(NBLK):
                    nc.sync.dma_start(out=bc[s][b][:], in_=db[b])
            expw = [cpool.tile([32, 128], bf16, name=f"expw_{v}") for v in range(2)]
            for v in range(2):
                nc.sync.dma_start(out=expw[v][:], in_=dEXP[v])
            idt = cpool.tile([128, 128], bf16, name="idt")
            nc.sync.dma_start(out=idt[:], in_=dIDT[:, :])
            mp = [cpool.tile([128, 64], bf16, name=f"mp_{h}") for h in range(2)]
            md = [cpool.tile([128, 8], bf16, name=f"md_{h}") for h in range(2)]
            for h in range(2):
                nc.sync.dma_start(out=mp[h][:], in_=dMP[h][:, :])
                nc.sync.dma_start(out=md[h][:], in_=dMD[h][:, :])
            pwt = [cpool.tile([128, 256], bf16, name=f"pwt_{g}") for g in range(NREG)]
            for g in range(NREG):
                nc.sync.dma_start(out=pwt[g][:], in_=dPW[g])
            pbt = [cpool.tile([128, 1], f32, name=f"pbt_{m}") for m in range(2)]
            for m in range(2):
                nc.sync.dma_start(out=pbt[m][:], in_=dPB[128 * m:128 * (m + 1), :])

            # ---- persistent activations ----
            pad = [qpool.tile([BLK, PADW, PADW], bf16, name=f"pad_{b}") for b in range(NBLK)]
            for b in range(NBLK):
                nc.gpsimd.memset(pad[b][:], 0.0)
            Q = [qpool.tile([128, N], bf16, name=f"Q_{r}") for r in range(NREG)]
            kpart = [qpool.tile([128, NT], f32, name=f"kpart_{r}") for r in range(NREG)]
            vks_sb = [qpool.tile([128, 128], bf16, name=f"vks_{r}") for r in range(NREG)]

            xbt = {p: [qpool.tile([128, N], bf16, name=f"x{p}_{k}")
                       for k in range(2)] for p in ("h", "l")}
            # nt-major order so pass1's first matmuls aren't waiting on the
            # tail of a p/k-major DMA stream
            for nt in range(NT):
                for p, dx in (("h", x_h), ("l", x_l)):
                    for k in range(2):
                        nc.sync.dma_start(
                            out=xbt[p][k][:, TN * nt:TN * (nt + 1)],
                            in_=dx[128 * k:128 * (k + 1), TN * nt:TN * (nt + 1)])

            def xb(p, k, nt):
                return xbt[p][k][:, TN * nt:TN * (nt + 1)]

            # (w, x) pairs for the compensated qkv product
            QKV_TERMS = (("h", "h"), ("h", "l"), ("l", "h"))

            # ================ pass 1: natural order -> padded image =========
            for b in range(NBLK):
                for nt in range(NT):
                    ps = ppool.tile([BLK, 8, WW], f32, name="ps1", tag="mm")
                    for k in range(2):
                        nc.tensor.matmul(
                            ps[:], w1h[k][:, BLK * b:BLK * (b + 1)],
                            xb("h", k, nt).rearrange("p (a c) -> p a c", c=WW),
                            start=(k == 0), stop=(k == 1))
                    nc.scalar.copy(out=pad[b][:, 2 + 8 * nt:10 + 8 * nt, 2:2 + WW], in_=ps[:])

            # ============ shared per-tile attention stage ====================
            def new_vkp(s_idx):
                return [ppool.tile([128, 128], f32, name=f"vkp_{s_idx}_{t}",
                                   tag="vk", bufs=2) for t in range(2)]

            def process_stage(s_idx, nt, ks, vs, vkp):
                """ks/vs: 2 bf16 [128,512] stage tiles (relu'd k / raw v)."""
                for t in range(2):
                    r = 2 * s_idx + t
                    nc.vector.reduce_sum(out=kpart[r][:, nt:nt + 1], in_=ks[t][:], axis=AX.X)
                for jj in range(4):
                    kT = spool.tile([128, 256], bf16, name="kT", tag="kT", bufs=2)
                    vT = spool.tile([128, 256], bf16, name="vT", tag="vT", bufs=2)
                    for t in range(2):
                        tp = ppool.tile([128, 128], bf16, name="tp", tag="mm")
                        nc.tensor.transpose(tp[:], ks[t][:, 128 * jj:128 * (jj + 1)], idt[:])
                        # vector copy (~215ns) not scalar (~500ns): the tp
                        # eviction latency gates the shared mm-tag PSUM
                        # rotation that qkv/conv matmuls also cycle through
                        nc.vector.tensor_copy(out=kT[:, 128 * t:128 * (t + 1)], in_=tp[:])
                        tp2 = ppool.tile([128, 128], bf16, name="tp2", tag="mm")
                        nc.tensor.transpose(tp2[:], vs[t][:, 128 * jj:128 * (jj + 1)], idt[:])
                        nc.vector.tensor_copy(out=vT[:, 128 * t:128 * (t + 1)], in_=tp2[:])
                    first = (nt == 0 and jj == 0)
                    last = (nt == NT - 1 and jj == 3)
                    for t in range(2):
                        nc.tensor.matmul(
                            vkp[t][:],
                            kT[:, 128 * t:128 * (t + 1)], vT[:, 128 * t:128 * (t + 1)],
                            start=first, stop=last)
                if nt == NT - 1:
                    for t in range(2):
                        nc.scalar.copy(out=vks_sb[2 * s_idx + t][:], in_=vkp[t][:])

            # ========== pass 2: separated order -> Q + id-scale k/v ==========
            vkp_id = new_vkp(0)
            for nt in range(NT):
                ks, vs = [None, None], [None, None]
                for j in range(6):
                    ps = ppool.tile([128, TN], f32, name="ps2", tag="mm")
                    for i, (pw_, px_) in enumerate(QKV_TERMS):
                        for k in range(2):
                            nc.tensor.matmul(ps[:], w2[pw_][k][:, 128 * j:128 * (j + 1)],
                                             xb(px_, k, nt),
                                             start=(i == 0 and k == 0),
                                             stop=(i == 2 and k == 1))
                    if j < 2:
                        nc.scalar.activation(out=Q[j][:, TN * nt:TN * (nt + 1)], in_=ps[:],
                                             func=AF.Relu, bias=bi2[j][:], scale=1.0)
                    elif j < 4:
                        t = j - 2
                        kst = spool.tile([128, TN], bf16, name="ks", tag=f"ks{t}", bufs=2)
                        nc.scalar.activation(out=kst[:], in_=ps[:], func=AF.Relu,
                                             bias=bi2[j][:], scale=1.0)
                        ks[t] = kst
                    else:
                        t = j - 4
                        vst = spool.tile([128, TN], bf16, name="vs", tag=f"vs{t}", bufs=2)
                        nc.vector.tensor_scalar(out=vst[:], in0=ps[:], scalar1=bi2[j][:],
                                                scalar2=None, op0=ALU.add)
                        vs[t] = vst
                process_stage(0, nt, ks, vs, vkp_id)

            # ================= fused conv scales =============================
            for s, taps, dfw, s_idx in ((3, TAPS3, dFW3, 1), (5, TAPS5, dFW5, 2)):
                vkp_s = new_vkp(s_idx)
                for h0 in range(0, NT, HALF):
                    stg = {}
                    for nth in range(HALF):
                        for t in range(2):
                            stg[("k", nth, t)] = spool.tile(
                                [128, TN], bf16, name="ks", tag=f"ks{t}", bufs=2)
                            stg[("v", nth, t)] = spool.tile(
                                [128, TN], bf16, name="vs", tag=f"vs{t}", bufs=2)
                    for b in range(NBLK):
                        # bufs=4: block b+1's accumulators must not wait on
                        # block b's PSUM eviction (was a 2.7us PE stall/block)
                        cps = [ppool.tile([BLK, 8, WW], f32, name="cp",
                                          tag="conv", bufs=4)
                               for _ in range(HALF)]
                        # fetch 2 taps per DMA trigger on the (otherwise idle)
                        # sync engine: per-(tap,block) gpsimd triggers used to
                        # occupy GpSimd ~660us, pacing the whole conv phase
                        npair = (len(taps) + 1) // 2
                        fwt2 = None
                        for ti, (dy, dx) in enumerate(taps):
                            if ti % 2 == 0:
                                fwt2 = wpool.tile([BLK, 2 * BLK], bf16,
                                                  name="fwt", tag="fw")
                                nc.sync.dma_start(
                                    out=fwt2[:], in_=dfw[b * npair + ti // 2])
                            fwt = fwt2[:, BLK * (ti % 2):BLK * (ti % 2 + 1)]
                            for nth in range(HALF):
                                nt = h0 + nth
                                nc.tensor.matmul(
                                    cps[nth][:], fwt,
                                    pad[b][:, 2 + 8 * nt + dy:10 + 8 * nt + dy,
                                           2 + dx:2 + dx + WW],
                                    start=(ti == 0), stop=(ti == len(taps) - 1))
                        qt, qr = (256 * s_idx + 32 * b) // 128, (32 * b) % 128
                        t2, r2 = b // 4, (32 * b) % 128
                        for nth in range(HALF):
                            nt = h0 + nth
                            cp = cps[nth]
                            nc.scalar.activation(
                                out=Q[qt][qr:qr + 32, TN * nt:TN * (nt + 1)],
                                in_=cp[0:32].rearrange("p a c -> p (a c)"),
                                func=AF.Relu, bias=bc[s][b][0:32, :], scale=1.0)
                            nc.scalar.activation(
                                out=stg[("k", nth, t2)][r2:r2 + 32, :],
                                in_=cp[32:64].rearrange("p a c -> p (a c)"),
                                func=AF.Relu, bias=bc[s][b][32:64, :], scale=1.0)
                            nc.vector.tensor_scalar(
                                out=stg[("v", nth, t2)][r2:r2 + 32, :],
                                in0=cp[64:96].rearrange("p a c -> p (a c)"),
                                scalar1=bc[s][b][64:96, :], scalar2=None, op0=ALU.add)
                    for nth in range(HALF):
                        process_stage(s_idx, h0 + nth,
                                      [stg[("k", nth, t)] for t in range(2)],
                                      [stg[("v", nth, t)] for t in range(2)], vkp_s)

            # ============== assemble apply weights from vk ===================
            apw2 = []
            denw = []
            for r in range(NREG):
                kf = qpool.tile([128, 1], f32, name=f"kfin_{r}")
                nc.vector.reduce_sum(out=kf[:], in_=kpart[r][:], axis=AX.X)
                vks = vks_sb[r]
                # den weights for quad-packed den matmuls: [128, 32] with this
                # region's two halves in columns 16*(r%2)..+16, zeros elsewhere
                dnw = qpool.tile([128, 32], bf16, name=f"denw_{r}")
                nc.gpsimd.memset(dnw[:], 0.0)
                # paired apply weights: both halves of the region in one
                # [128, 128] lhsT (one apply matmul per region per tile)
                aw2 = qpool.tile([128, 128], bf16, name=f"apw2_{r}")
                for half in range(2):
                    nc.vector.tensor_tensor(
                        out=aw2[:, 64 * half:64 * (half + 1)].rearrange(
                            "p (d h) -> p d h", h=8),
                        in0=vks[:, 64 * half:64 * (half + 1)].rearrange(
                            "p (h d) -> p d h", d=8),
                        in1=mp[half][:].rearrange("p (d h) -> p d h", h=8),
                        op=ALU.mult)
                    nc.vector.tensor_scalar(
                        out=dnw[:, 16 * (r % 2) + 8 * half:16 * (r % 2) + 8 * half + 8],
                        in0=md[half][:], scalar1=kf[:], scalar2=None, op0=ALU.mult)
                apw2.append(aw2)
                denw.append(dnw)

            # ================= apply + normalize + proj ======================
            for nt in range(NT):
                pjs = [ppool.tile([128, TN], f32, name=f"pj{m}", tag="conv", bufs=4)
                       for m in range(2)]
                # pass A: all 12 denominators -> one batched reciprocal
                # (per-group [8,512] reciprocal chains cost ~190us of vector
                # time and serialized against PE)
                den12 = spool.tile([96, TN], f32, name="den12", tag="den", bufs=2)
                for G in range(3):
                    # reuse the vk PSUM banks (dead after vk assembly): with
                    # tag "mm" the first den matmul of tile nt stalls ~3us on
                    # tile nt-1's aps/eps buffers draining
                    dps = ppool.tile([32, TN], f32, name="dps", tag="vk", bufs=2)
                    for rr in range(2):
                        r = 2 * G + rr
                        nc.tensor.matmul(dps[:], denw[r][:],
                                         Q[r][:, TN * nt:TN * (nt + 1)],
                                         start=(rr == 0), stop=(rr == 1))
                    nc.scalar.copy(out=den12[32 * G:32 * (G + 1), :], in_=dps[:])
                nc.vector.tensor_scalar(out=den12[:], in0=den12[:], scalar1=1e-15,
                                        scalar2=None, op0=ALU.add)
                rc12 = spool.tile([96, TN], f32, name="rc12", tag="rc", bufs=1)
                scr12 = spool.tile([96, TN], f32, name="scr12", tag="scr", bufs=1)
                nc.vector.reciprocal_approx_accurate(out=rc12[:], in_=den12[:],
                                                     scratch=scr12[:])
                # three base-0 tiles: matmul rhs must share base partition
                # with its lhsT (expw variants live at base 0)
                rcb32 = []
                for G in range(3):
                    rt = spool.tile([32, TN], bf16, name=f"rcb{G}", tag="rcb", bufs=3)
                    nc.scalar.copy(out=rt[:], in_=rc12[32 * G:32 * (G + 1), :])
                    rcb32.append(rt)
                # pass B: apply -> normalize -> proj, both halves of a region
                # paired into single [128,128]-lhsT matmuls
                for j in range(NREG):
                    aps = ppool.tile([128, TN], f32, name="aps", tag="mm")
                    nc.tensor.matmul(aps[:], apw2[j][:], Q[j][:, TN * nt:TN * (nt + 1)],
                                     start=True, stop=True)
                    eps = ppool.tile([128, TN], f32, name="eps", tag="mm")
                    nc.tensor.matmul(eps[:], expw[j % 2][:], rcb32[j // 2][:],
                                     start=True, stop=True)
                    exb = spool.tile([128, TN], f32, name="exb", tag="exb", bufs=3)
                    nc.scalar.copy(out=exb[:], in_=eps[:])
                    at = spool.tile([128, TN], bf16, name="at", tag="at", bufs=3)
                    nc.vector.tensor_tensor(out=at[:], in0=aps[:], in1=exb[:], op=ALU.mult)
                    for m in range(2):
                        nc.tensor.matmul(pjs[m][:], pwt[j][:, 128 * m:128 * (m + 1)],
                                         at[:], start=(j == 0), stop=(j == NREG - 1))
                for m in range(2):
                    ob = spool.tile([128, TN], f32, name="ob", tag="ob", bufs=2)
                    nc.vector.tensor_scalar(out=ob[:], in0=pjs[m][:], scalar1=pbt[m][:],
                                            scalar2=None, op0=ALU.add)
                    nc.sync.dma_start(
                        out=d_out[128 * m:128 * (m + 1), TN * nt:TN * (nt + 1)], in_=ob[:])
    return nc


def _get_nc():
    if "nc" not in _cache:
        nc = _build()
        nc.compile()
        _cache["nc"] = nc
    return _cache["nc"]


def _whash(inputs):
    h = hashlib.blake2b(digest_size=16)
    for name in ("qkv_w", "qkv_b", "dw3_w", "dw3_b", "pw3_w", "pw3_b",
                 "dw5_w", "dw5_b", "pw5_w", "pw5_b", "proj_w", "proj_b"):
        h.update(np.ascontiguousarray(np.asarray(inputs[name], np.float32)))
    return h.hexdigest()


def _feeds(inputs):
    import ml_dtypes

    def bf(a):
        return np.asarray(a, ml_dtypes.bfloat16)

    def split(a):
        hi = bf(a)
        lo = bf(np.asarray(a, np.float32) - np.asarray(hi, np.float32))
        return hi, lo

    key = _whash(inputs)
    if _cache.get("feeds_key") != key:
        d = _host_weights(inputs)
        w2h, w2l = split(d["w2t"])
        base = {
            "w1h": bf(d["w1t"]), "w2h": w2h, "w2l": w2l,
            "bi2": d["bi2"].astype(np.float32),
            "fw3": bf(d["fw3"]), "fw5": bf(d["fw5"]),
            "bc3": d["bc3"].astype(np.float32), "bc5": d["bc5"].astype(np.float32),
            "exp": bf(d["exp"]), "idt": bf(d["idt"]),
            "mp0": bf(d["mp0"]), "mp1": bf(d["mp1"]),
            "md0": bf(d["md0"]), "md1": bf(d["md1"]),
            "pw": bf(d["pw"]), "pb": d["pb"].astype(np.float32),
        }
        _cache["feeds_key"] = key
        _cache["feeds_base"] = base
    x = np.asarray(inputs["x"], np.float32).reshape(B, CIN, N)
    xh, xl = split(x)
    return _cache["feeds_base"], (xh, xl)


def _get_runner():
    """Build the jitted shard_map callable once; reuse across kernel() calls.

    Mirrors concourse.bass2jax.run_bass_via_pjrt but caches the jitted
    function (avoids re-lowering/re-compiling the XLA wrapper per call) and
    keeps the replicated weight operands device-resident.
    """
    if "runner" in _cache:
        return _cache["runner"]
    import jax
    import concourse.mybir as mybir
    from concourse import bass2jax
    from jax.experimental.shard_map import shard_map
    from jax.sharding import Mesh, PartitionSpec

    bass2jax.install_neuronx_cc_hook()
    nc = _get_nc()
    assert nc.dbg_addr is None or not nc.dbg_callbacks

    partition_name = (nc.partition_id_tensor.name
                      if nc.partition_id_tensor else None)
    in_names, out_names, out_avals = [], [], []
    for alloc in nc.m.functions[0].allocations:
        if not isinstance(alloc, mybir.MemoryLocationSet):
            continue
        name = alloc.memorylocations[0].name
        if alloc.kind == "ExternalInput":
            if name != partition_name:
                in_names.append(name)
        elif alloc.kind == "ExternalOutput":
            out_names.append(name)
            out_avals.append(jax.core.ShapedArray(
                tuple(alloc.tensor_shape), mybir.dt.np(alloc.dtype)))
    n_params = len(in_names)
    all_in = in_names + out_names + ([partition_name] if partition_name else [])

    def _body(*args):
        operands = list(args)
        if partition_name is not None:
            operands.append(bass2jax.partition_id_tensor())
        return tuple(bass2jax._bass_exec_p.bind(
            *operands,
            out_avals=tuple(out_avals),
            in_names=tuple(all_in),
            out_names=tuple(out_names),
            lowering_input_output_aliases=(),
            sim_require_finite=True,
            sim_require_nnan=True,
            nc=nc,
        ))

    devices = jax.devices()[:B]
    mesh = Mesh(np.asarray(devices), ("core",))
    sharded = jax.jit(
        shard_map(_body, mesh=mesh,
                  in_specs=(PartitionSpec("core"),) * (n_params + len(out_names)),
                  out_specs=(PartitionSpec("core"),) * len(out_names),
                  check_rep=False),
        keep_unused=True)
    _cache["runner"] = (sharded, in_names, out_names, out_avals, mesh)
    return _cache["runner"]


def kernel(**inputs):
    import jax
    from jax.sharding import NamedSharding, PartitionSpec

    base, (xh, xl) = _feeds(inputs)
    sharded, in_names, out_names, out_avals, mesh = _get_runner()

    sh = NamedSharding(mesh, PartitionSpec("core"))
    key = _cache["feeds_key"]
    if _cache.get("dev_key") != key:
        dev = {}
        for name in in_names:
            if name in ("xh", "xl"):
                continue
            a = np.asarray(base[name])
            rep = np.concatenate([a] * B, axis=0)
            dev[name] = jax.device_put(rep, sh)
        _cache["dev_key"] = key
        _cache["dev_weights"] = dev
    dev = _cache["dev_weights"]

    if "dev_zeros" not in _cache:
        _cache["dev_zeros"] = [
            jax.device_put(
                np.zeros((B * av.shape[0],) + tuple(av.shape[1:]), av.dtype), sh)
            for av in out_avals]

    xg = {"xh": xh, "xl": xl}
    args = []
    for name in in_names:
        if name in ("xh", "xl"):
            args.append(np.ascontiguousarray(xg[name].reshape(B * CIN, N)))
        else:
            args.append(dev[name])
    args.extend(_cache["dev_zeros"])

    out_arrs = sharded(*args)
    idx = out_names.index("out")
    out = np.asarray(out_arrs[idx]).reshape(B, CIN, HH, WW)
    return out.astype(np.float32)

